# revision 1
# baseline (speedup 1.0000x reference)
"""Bipartite GCN stack (2 layers) on 8 Trainium2 NeuronCores.

Strategy (graph/data parallel, destination-sharded):
  - Layer-0 forward: every core computes the full WH0 = H_src @ W_fwd0
    (redundantly; cheaper than all-gathering the 64MB table), then
    processes the ~1/8 of edges whose destination (row) falls in its
    4096-target shard.  segment_sum is done by gathering WH0 rows in
    1024-row batches with the Q7 dma_gather instruction and reducing
    each 128-edge tile into PSUM with a selection-matrix matmul
    (S[e,d] = val[e] if dst_local[e]==d).  A second tiny matmul against
    a ones column accumulates the degrees.
  - dma_gather needs int16 indices, so 65536-row tables are split into
    lo/hi halves of 32768 rows and each destination tile's edges are
    grouped by half (host-side index preprocessing only).
  - BatchNorm: per-core partial sums via bn_stats in feature-major
    layout, 2KB AllReduce, scale/shift applied in feature-major form
    (which is exactly the lhsT layout the next dense matmul needs).
  - Backward (layer 0 only; the layer-1 backward output is dead):
    AllGather of WHb, edges sharded by source, same reduction.
  - Layer-1 forward: AllGather of WH1, reuse of the layer-0 target
    degrees, residual in feature-major form, BN, output.

Gathered tables are stored in _TABLE_DT (bfloat16 by default: 4x PE
matmul rate, half the gather/collective bytes); everything else is
fp32.  Host-side work is limited to sharding/permutation of the edge
index arrays and layout transforms of inputs/outputs; all
floating-point math runs on the NeuronCores.
"""

import numpy as np

P = 128
D_FIXED = 256
EPS = 1e-5
NCORES = 8
GBT = 8  # gather batch: edge tiles per dma_gather (1024 rows)

# dtype of the gathered feature tables (WH0/WHb/WH1), the gather buffers
# and the selection matrices: "float32" (exact) or "bfloat16"
_TABLE_DT = "bfloat16"


# ----------------------------------------------------------------- host prep


def _edge_plan(dst, gidx, vals, n_dst, n_gather, ncores, split):
    """Partition edges by destination shard, group by 128-row dst tile and
    (optionally) by gather-table half; pad each (core, tile[, half]) group
    to a multiple of 128 edges, common across cores.

    Returns:
      segs:   per dst-tile, list of (half, ntiles)
      idx16:  per-core [128, NF*8] int16 gather indices (16-partition
              wrapped layout for dma_gather, replicated 8x vertically)
      val:    per-core [P, NF] f32
      dl:     per-core [P, NF] f32 (dst_local in 0..127)
    """
    dst_sh = n_dst // ncores
    nt = dst_sh // P
    nhalf = 2 if split else 1
    half_rows = n_gather // nhalf

    core_of = dst // dst_sh
    tile_of = (dst % dst_sh) // P
    dl_of = (dst % P).astype(np.float32)
    half_of = (gidx // half_rows) if split else np.zeros(len(dst), np.int64)
    lidx = (gidx - half_of * half_rows).astype(np.int16)

    grp = (core_of * nt + tile_of) * nhalf + half_of
    order = np.lexsort((gidx, grp))
    so_lidx = lidx[order]
    so_val = vals[order].astype(np.float32)
    so_dl = dl_of[order]

    ngrp = ncores * nt * nhalf
    counts = np.bincount(grp, minlength=ngrp).reshape(ncores, nt, nhalf)
    # per (tile, half) tile count, common across cores; >=1 tile per dst tile
    ntile_th = np.ceil(counts.max(axis=0) / P).astype(np.int64)  # [nt, nhalf]
    for t in range(nt):
        if ntile_th[t].sum() == 0:
            ntile_th[t, 0] = 1
    nf = int(ntile_th.sum())
    off_flat = np.concatenate([[0], np.cumsum(ntile_th.reshape(-1))])

    i_arr = np.zeros((ncores, nf * P), dtype=np.int16)
    v_arr = np.zeros((ncores, nf * P), dtype=np.float32)
    d_arr = np.zeros((ncores, nf * P), dtype=np.float32)

    grp_start = np.concatenate([[0], np.cumsum(counts.reshape(-1))])
    for c in range(ncores):
        for t in range(nt):
            for h in range(nhalf):
                gi = (c * nt + t) * nhalf + h
                s, e = grp_start[gi], grp_start[gi + 1]
                n = e - s
                o = off_flat[t * nhalf + h] * P
                i_arr[c, o : o + n] = so_lidx[s:e]
                v_arr[c, o : o + n] = so_val[s:e]
                d_arr[c, o : o + n] = so_dl[s:e]

    segs = [
        [(h, int(ntile_th[t, h])) for h in range(nhalf) if ntile_th[t, h] > 0]
        for t in range(nt)
    ]
    # dma_gather index layout: linear idx i at [i % 16, i // 16], x8 vertical
    idx16 = []
    for c in range(ncores):
        a = i_arr[c].reshape(nf * 8, 16).T  # [16, nf*8]
        idx16.append(np.ascontiguousarray(np.tile(a, (8, 1))))
    v_dev = [np.ascontiguousarray(v_arr[c].reshape(nf, P).T) for c in range(ncores)]
    d_dev = [np.ascontiguousarray(d_arr[c].reshape(nf, P).T) for c in range(ncores)]
    return segs, idx16, v_dev, d_dev


# ----------------------------------------------------------------- bass build


def _install_drain_patch():
    """walrus in this env allows only ONE sem-wait per instruction; split
    extra waits onto same-engine carrier instructions."""
    import concourse.mybir as mybir
    import concourse.tile as _tile
    from concourse.vector_clock import ScopedClock

    if getattr(_tile.TileContext, "_drain_split_patched", False):
        return

    def _split_drain_and_barrier(self, tick_clock, wait_clock):
        nc = self.nc
        drain_inst = nc.sync.drain()
        wait_clock.add_sem_waits(
            drain_inst.ins, ScopedClock({None: tick_clock.global_clock})
        )
        si = drain_inst.ins.sync_info
        waits = list(si.on_wait) if si and si.on_wait else []
        if len(waits) > 1:
            si.on_wait = waits[:1]
            drain_inst.ins.sync_info = si
            for i in range(1, len(waits)):
                extra = nc.sync.drain()
                esi = extra.ins.sync_info
                upd = list(esi.on_update) if esi and esi.on_update else []
                extra.ins.sync_info = mybir.SyncInfo(
                    on_wait=[waits[i]], on_update=upd
                )
        nc.all_engine_barrier()
        assert self.sems is not None
        popped = nc._tile_sem_poison_stack.pop()
        assert popped is self._sem_poison
        nc.clear_and_free_semaphores(list(self.sems.allocated().values()))
        nc.all_engine_barrier()

    _tile.TileContext._drain_and_barrier = _split_drain_and_barrier

    _orig_add = _tile.TileContext._add_instruction

    def _add_instruction_split(self, inst):
        si = inst.sync_info
        waits = list(si.on_wait) if si and si.on_wait else []
        if len(waits) > 1 and inst.engine != mybir.EngineType.Unassigned:
            for w in waits[:-1]:
                nop = mybir.InstNoOp(
                    name=self.nc.get_next_instruction_name(), ins=[], outs=[]
                )
                nop.engine = inst.engine
                nop.sync_info = mybir.SyncInfo(on_wait=[w], on_update=[])
                _orig_add(self, nop)
            si.on_wait = waits[-1:]
            inst.sync_info = si
        _orig_add(self, inst)

    _tile.TileContext._add_instruction = _add_instruction_split
    _tile.TileContext._drain_split_patched = True


def _build_program(n_tgt, n_src, fsegs, bsegs, taps=False):
    """Build the SPMD bass program (identical on all 8 cores)."""
    from contextlib import ExitStack

    import concourse.bass as bass
    import concourse.mybir as mybir
    import concourse.tile as tile
    from concourse import bacc
    from concourse.masks import make_identity

    _install_drain_patch()

    dt = mybir.dt
    f32 = dt.float32
    i16 = dt.int16
    tb = getattr(dt, _TABLE_DT)
    D = D_FIXED
    DC = D // P
    tgt_sh = n_tgt // NCORES
    src_sh = n_src // NCORES
    NT = tgt_sh // P
    NF = sum(n for seg in fsegs for _, n in seg)
    NB = sum(n for seg in bsegs for _, n in seg)
    AluOp = mybir.AluOpType
    Act = mybir.ActivationFunctionType
    rg = [list(range(NCORES))]

    nc = bacc.Bacc("TRN2", target_bir_lowering=False, debug=False, num_devices=NCORES)

    dram_t = nc.dram_tensor
    HsrcT = dram_t("HsrcT", [D, n_src], f32, kind="ExternalInput").ap()
    W0 = dram_t("W0", [D, D], f32, kind="ExternalInput").ap()
    Wb = dram_t("Wb", [D, D], f32, kind="ExternalInput").ap()
    W1 = dram_t("W1", [D, D], f32, kind="ExternalInput").ap()
    b0_h = dram_t("b0", [1, D], f32, kind="ExternalInput")
    bb_h = dram_t("bb", [1, D], f32, kind="ExternalInput")
    b1_h = dram_t("b1", [1, D], f32, kind="ExternalInput")
    g1T = dram_t("g1T", [P, DC], f32, kind="ExternalInput").ap()
    be1T = dram_t("be1T", [P, DC], f32, kind="ExternalInput").ap()
    g2T = dram_t("g2T", [P, DC], f32, kind="ExternalInput").ap()
    be2T = dram_t("be2T", [P, DC], f32, kind="ExternalInput").ap()
    iota_d = dram_t("iota", [P, P], f32, kind="ExternalInput").ap()
    emb = dram_t("emb", [tgt_sh, D], f32, kind="ExternalInput").ap()
    fe_i16 = dram_t("fe_i16", [P, NF * 8], i16, kind="ExternalInput").ap()
    fe_val = dram_t("fe_val", [P, NF], f32, kind="ExternalInput").ap()
    fe_dl = dram_t("fe_dl", [P, NF], f32, kind="ExternalInput").ap()
    be_i16 = dram_t("be_i16", [P, NB * 8], i16, kind="ExternalInput").ap()
    be_val = dram_t("be_val", [P, NB], f32, kind="ExternalInput").ap()
    be_dl = dram_t("be_dl", [P, NB], f32, kind="ExternalInput").ap()
    outT = dram_t("outT", [D, tgt_sh], f32, kind="ExternalOutput").ap()
    if taps:
        dbg_wh0 = dram_t("dbg_wh0", [n_src, D], tb, kind="ExternalOutput").ap()
        dbg_x1T = dram_t("dbg_x1T", [D, tgt_sh], f32, kind="ExternalOutput").ap()
        dbg_whb = dram_t("dbg_whb", [n_tgt, D], tb, kind="ExternalOutput").ap()
        dbg_wh1 = dram_t("dbg_wh1", [n_src, D], tb, kind="ExternalOutput").ap()
        dbg_st1 = dram_t("dbg_st1", [P, 4], f32, kind="ExternalOutput").ap()
        dbg_x1pre = dram_t("dbg_x1pre", [D, tgt_sh], f32, kind="ExternalOutput").ap()
        dbg_deg = dram_t("dbg_deg", [P, NT], f32, kind="ExternalOutput").ap()

    with tile.TileContext(nc) as tc, ExitStack() as ctx:
        dram = ctx.enter_context(tc.tile_pool(name="dram", bufs=1, space="DRAM"))
        half_rows = n_src // 2
        WH0_t = [dram.tile([half_rows, D], tb, name=f"WH0h{h}") for h in range(2)]
        WHb_loc = dram.tile([tgt_sh, D], tb)
        WHb_full = dram.tile([n_tgt, D], tb, addr_space="Shared")
        WH1_loc = dram.tile([src_sh, D], tb)
        WH1_full = dram.tile([n_src, D], tb, addr_space="Shared")
        WH1_t = [
            WH1_full[h * half_rows : (h + 1) * half_rows, :] for h in range(2)
        ]
        st1_in = dram.tile([P, 2 * DC], f32)
        st1_out = dram.tile([P, 2 * DC], f32, addr_space="Shared")
        st2_in = dram.tile([P, 2 * DC], f32)
        st2_out = dram.tile([P, 2 * DC], f32, addr_space="Shared")

        consts = ctx.enter_context(tc.tile_pool(name="consts", bufs=1))
        w0t = consts.tile([P, DC, D], f32)
        wbt = consts.tile([P, DC, D], f32)
        w1t = consts.tile([P, DC, D], f32)
        for c in range(DC):
            nc.sync.dma_start(out=w0t[:, c, :], in_=W0[c * P : (c + 1) * P, :])
            nc.sync.dma_start(out=wbt[:, c, :], in_=Wb[c * P : (c + 1) * P, :])
            nc.sync.dma_start(out=w1t[:, c, :], in_=W1[c * P : (c + 1) * P, :])
        w0b = consts.tile([P, DC, D], tb)
        wbb = consts.tile([P, DC, D], tb)
        for c in range(DC):
            nc.vector.tensor_copy(out=w0b[:, c, :], in_=w0t[:, c, :])
            nc.vector.tensor_copy(out=wbb[:, c, :], in_=wbt[:, c, :])
        b0bc = consts.tile([P, D], f32)
        bbbc = consts.tile([P, D], f32)
        b1bc = consts.tile([P, D], f32)
        for h_, t_ in ((b0_h, b0bc), (bb_h, bbbc), (b1_h, b1bc)):
            nc.gpsimd.dma_start(
                out=t_[:], in_=bass.AP(tensor=h_, offset=0, ap=[[0, P], [1, D]])
            )
        g1f = consts.tile([P, DC], f32)
        be1f = consts.tile([P, DC], f32)
        g2f = consts.tile([P, DC], f32)
        be2f = consts.tile([P, DC], f32)
        nc.sync.dma_start(out=g1f[:], in_=g1T[:])
        nc.sync.dma_start(out=be1f[:], in_=be1T[:])
        nc.sync.dma_start(out=g2f[:], in_=g2T[:])
        nc.sync.dma_start(out=be2f[:], in_=be2T[:])
        iota_t = consts.tile([P, P], f32)
        nc.sync.dma_start(out=iota_t[:], in_=iota_d[:])
        epst = consts.tile([P, 1], f32)
        nc.vector.memset(epst[:], EPS)
        ident = consts.tile([P, P], f32)
        make_identity(nc, ident[:])
        onesb = consts.tile([P, 1], tb)
        nc.vector.memset(onesb[:], 1.0)

        # resident state
        xT = consts.tile([P, DC, NT, P], tb)  # H_tgt1 (feature-major)
        x2T = consts.tile([P, DC, NT, P], f32)  # layer-1 pre-BN x
        degc = consts.tile([P, NT], f32)  # reciprocal clamped target degree

        sv_pool = ctx.enter_context(tc.tile_pool(name="sv", bufs=8))
        g_pool = ctx.enter_context(tc.tile_pool(name="gp", bufs=3))
        ps_acc = ctx.enter_context(tc.tile_pool(name="psacc", bufs=3, space="PSUM"))
        ps_tr = ctx.enter_context(tc.tile_pool(name="pstr", bufs=2, space="PSUM"))
        hrm = ctx.enter_context(tc.tile_pool(name="hrm", bufs=4))
        whs_pool = ctx.enter_context(tc.tile_pool(name="whs", bufs=4))
        lhs_pool = ctx.enter_context(tc.tile_pool(name="lhs", bufs=2))
        misc = ctx.enter_context(tc.tile_pool(name="misc", bufs=6))

        # ---------------- phase A: full WH0 = H_src @ W0 (every core) ----
        SUP = 512 if n_src % 512 == 0 else P
        for st in range(n_src // SUP):
            haf = lhs_pool.tile([P, SUP], f32, tag="haf")
            hbf = lhs_pool.tile([P, SUP], f32, tag="hbf")
            nc.sync.dma_start(out=haf[:], in_=HsrcT[0:P, st * SUP : (st + 1) * SUP])
            nc.sync.dma_start(
                out=hbf[:], in_=HsrcT[P : 2 * P, st * SUP : (st + 1) * SUP]
            )
            ha = lhs_pool.tile([P, SUP], tb, tag="ha")
            hb = lhs_pool.tile([P, SUP], tb, tag="hb")
            nc.vector.tensor_copy(out=ha[:], in_=haf[:])
            nc.vector.tensor_copy(out=hb[:], in_=hbf[:])
            for r in range(SUP // P):
                ps = ps_acc.tile([P, D], f32, tag="acc")
                nc.tensor.matmul(
                    out=ps[:],
                    lhsT=ha[:, r * P : (r + 1) * P],
                    rhs=w0b[:, 0, :],
                    start=True,
                    stop=False,
                )
                nc.tensor.matmul(
                    out=ps[:],
                    lhsT=hb[:, r * P : (r + 1) * P],
                    rhs=w0b[:, 1, :],
                    start=False,
                    stop=True,
                )
                whs = whs_pool.tile([P, D], tb, tag="whs")
                nc.scalar.copy(out=whs[:], in_=ps[:])
                row0 = (st * (SUP // P) + r) * P
                h = row0 // half_rows
                lr = row0 - h * half_rows
                nc.sync.dma_start(out=WH0_t[h][lr : lr + P, :], in_=whs[:])

        # ---------------- gather pass helper ----------------------------
        def gather_pass(segs, idx_tile, val_tile, dl_tile, tables, want_deg, post):
            """segs: per dst-tile list of (table_idx, ntiles)."""
            j = 0  # global edge-tile index
            for t in range(len(segs)):
                ntile_tot = sum(n for _, n in segs[t])
                ps = ps_acc.tile([P, D], f32, tag="acc", name=f"ps{t}")
                psd = None
                if want_deg:
                    psd = ps_acc.tile([P, 1], f32, tag="deg", bufs=2, name=f"psd{t}")
                k = 0  # tile index within dst-tile
                for tab_i, nseg in segs[t]:
                    table = tables[tab_i]
                    done = 0
                    while done < nseg:
                        bs = min(GBT, nseg - done)
                        gt = g_pool.tile([P, GBT, D], tb, tag="gt", name="gt")
                        nc.gpsimd.dma_gather(
                            out_ap=gt[:, 0:bs, :],
                            in_ap=table,
                            idxs_ap=idx_tile[:, j * 8 : (j + bs) * 8],
                            num_idxs=bs * P,
                            num_idxs_reg=bs * P,
                            elem_size=D,
                            single_packet=False,
                        )
                        for i in range(bs):
                            jj = j + i
                            sv = sv_pool.tile([P, P], tb, name="sv")
                            nc.vector.scalar_tensor_tensor(
                                out=sv[:],
                                in0=iota_t[:],
                                scalar=dl_tile[:, jj : jj + 1],
                                in1=val_tile[:, jj : jj + 1].to_broadcast([P, P]),
                                op0=AluOp.is_equal,
                                op1=AluOp.mult,
                            )
                            nc.tensor.matmul(
                                out=ps[:],
                                lhsT=sv[:],
                                rhs=gt[:, i, :],
                                start=(k + i == 0),
                                stop=(k + i == ntile_tot - 1),
                            )
                            if psd is not None:
                                nc.tensor.matmul(
                                    out=psd[:],
                                    lhsT=sv[:],
                                    rhs=onesb[:],
                                    start=(k + i == 0),
                                    stop=(k + i == ntile_tot - 1),
                                )
                        j += bs
                        done += bs
                        k += bs
                post(t, ps, psd)
            return j

        # ---------------- pass B: layer-0 forward ------------------------
        def post_fwd0(t, ps, psd):
            dtmp = misc.tile([P, 1], f32, tag="dtgt")
            nc.vector.tensor_scalar_max(dtmp[:], psd[:], 1.0)
            nc.vector.reciprocal(degc[:, t : t + 1], dtmp[:])
            h1 = hrm.tile([P, D], f32, tag="h1")
            nc.vector.scalar_tensor_tensor(
                out=h1[:],
                in0=ps[:],
                scalar=degc[:, t : t + 1],
                in1=b0bc[:],
                op0=AluOp.mult,
                op1=AluOp.add,
            )
            h2 = hrm.tile([P, D], f32, tag="h2")
            nc.scalar.activation(out=h2[:], in_=h1[:], func=Act.Relu)
            et = misc.tile([P, D], f32, tag="emb")
            nc.sync.dma_start(out=et[:], in_=emb[t * P : (t + 1) * P, :])
            xr = hrm.tile([P, D], f32, tag="xr")
            nc.vector.tensor_add(xr[:], h2[:], et[:])
            for c in range(DC):
                pt = ps_tr.tile([P, P], f32, tag="tr")
                nc.tensor.transpose(
                    out=pt[:], in_=xr[:, c * P : (c + 1) * P], identity=ident[:]
                )
                nc.vector.tensor_copy(out=xT[:, c, t, :], in_=pt[:])

        with tc.tile_pool(name="edgesB", bufs=1) as ep:
            fidx = ep.tile([P, NF * 8], i16, name="fidxB")
            fval = ep.tile([P, NF], f32, name="fvalB")
            fdl = ep.tile([P, NF], f32, name="fdlB")
            nc.sync.dma_start(out=fidx[:], in_=fe_i16[:])
            nc.sync.dma_start(out=fval[:], in_=fe_val[:])
            nc.sync.dma_start(out=fdl[:], in_=fe_dl[:])
            gather_pass(
                fsegs, fidx, fval, fdl, [t_[:] for t_ in WH0_t], True, post_fwd0
            )

        # ---------------- BN helpers -------------------------------------
        def bn_stats_phase(xbuf, count, st_in_sb_name):
            st_sb = misc.tile([P, 2 * DC], f32, name=st_in_sb_name, tag="stats")
            grp = min(512, count)
            ngrp = count // grp
            for c in range(DC):
                bnst = misc.tile([P, ngrp, 6], f32, tag="bnst")
                flat = xbuf[:, c, :, :].rearrange("p a b -> p (a b)")
                for g in range(ngrp):
                    nc.vector.bn_stats(
                        out=bnst[:, g, :], in_=flat[:, g * grp : (g + 1) * grp]
                    )
                mv = misc.tile([P, 2], f32, tag="mv")
                nc.vector.bn_aggr(out=mv[:], in_=bnst[:].rearrange("p a b -> p (a b)"))
                nc.vector.tensor_scalar_mul(
                    st_sb[:, 2 * c : 2 * c + 1], mv[:, 0:1], float(count)
                )
                musq = misc.tile([P, 1], f32, tag="musq")
                nc.vector.tensor_mul(musq[:], mv[:, 0:1], mv[:, 0:1])
                nc.vector.tensor_add(musq[:], musq[:], mv[:, 1:2])
                nc.vector.tensor_scalar_mul(
                    st_sb[:, 2 * c + 1 : 2 * c + 2], musq[:], float(count)
                )
            return st_sb

        def bn_coeffs(st_full_sb, gamma_f, beta_f, total, a_name, b_name):
            A = misc.tile([P, DC], f32, name=a_name, tag="bnA")
            B = misc.tile([P, DC], f32, name=b_name, tag="bnB")
            for c in range(DC):
                mu = misc.tile([P, 1], f32, tag="mu")
                nc.vector.tensor_scalar_mul(
                    mu[:], st_full_sb[:, 2 * c : 2 * c + 1], 1.0 / total
                )
                q = misc.tile([P, 1], f32, tag="q")
                nc.vector.tensor_scalar_mul(
                    q[:], st_full_sb[:, 2 * c + 1 : 2 * c + 2], 1.0 / total
                )
                musq = misc.tile([P, 1], f32, tag="musq2")
                nc.vector.tensor_mul(musq[:], mu[:], mu[:])
                var = misc.tile([P, 1], f32, tag="var")
                nc.vector.tensor_tensor(
                    out=var[:], in0=q[:], in1=musq[:], op=AluOp.subtract
                )
                sd = misc.tile([P, 1], f32, tag="sd")
                nc.scalar.activation(out=sd[:], in_=var[:], func=Act.Sqrt, bias=epst[:])
                rstd = misc.tile([P, 1], f32, tag="rstd")
                nc.vector.reciprocal(rstd[:], sd[:])
                nc.vector.tensor_mul(A[:, c : c + 1], gamma_f[:, c : c + 1], rstd[:])
                mA = misc.tile([P, 1], f32, tag="mA")
                nc.vector.tensor_mul(mA[:], mu[:], A[:, c : c + 1])
                nc.vector.tensor_tensor(
                    out=B[:, c : c + 1],
                    in0=beta_f[:, c : c + 1],
                    in1=mA[:],
                    op=AluOp.subtract,
                )
            return A, B

        if taps:
            for h in range(2):
                nc.sync.dma_start(
                    out=dbg_wh0[h * half_rows : (h + 1) * half_rows, :],
                    in_=WH0_t[h][:],
                )
            nc.sync.dma_start(out=dbg_deg[:], in_=degc[:])
            for c in range(DC):
                nc.sync.dma_start(
                    out=dbg_x1pre[c * P : (c + 1) * P, :],
                    in_=xT[:, c, :, :].rearrange("p a b -> p (a b)"),
                )

        # ---------------- BN-1 + WHb + AllGather -------------------------
        st1_sb = bn_stats_phase(xT, tgt_sh, "st1_sb")
        nc.sync.dma_start(out=st1_in[:], in_=st1_sb[:])
        nc.gpsimd.collective_compute(
            "AllReduce",
            AluOp.add,
            replica_groups=rg,
            ins=[st1_in[:].opt()],
            outs=[st1_out[:].opt()],
        )
        st1g = misc.tile([P, 2 * DC], f32, tag="stg")
        nc.sync.dma_start(out=st1g[:], in_=st1_out[:])
        A1, B1 = bn_coeffs(st1g, g1f, be1f, n_tgt, "A1", "B1")

        for t in range(NT):
            for c in range(DC):
                nc.vector.scalar_tensor_tensor(
                    out=xT[:, c, t, :],
                    in0=xT[:, c, t, :],
                    scalar=A1[:, c : c + 1],
                    in1=B1[:, c : c + 1].to_broadcast([P, P]),
                    op0=AluOp.mult,
                    op1=AluOp.add,
                )
            ps = ps_acc.tile([P, D], f32, tag="acc", name=f"pswb{t}")
            nc.tensor.matmul(
                out=ps[:], lhsT=xT[:, 0, t, :], rhs=wbb[:, 0, :], start=True, stop=False
            )
            nc.tensor.matmul(
                out=ps[:], lhsT=xT[:, 1, t, :], rhs=wbb[:, 1, :], start=False, stop=True
            )
            whs = whs_pool.tile([P, D], tb, tag="whs")
            nc.scalar.copy(out=whs[:], in_=ps[:])
            nc.sync.dma_start(out=WHb_loc[t * P : (t + 1) * P, :], in_=whs[:])

        nc.gpsimd.collective_compute(
            "AllGather",
            AluOp.bypass,
            replica_groups=rg,
            ins=[WHb_loc[:].opt()],
            outs=[WHb_full[:].opt()],
        )
        if taps:
            nc.sync.dma_start(out=dbg_st1[:], in_=st1_out[:])
            for c in range(DC):
                nc.sync.dma_start(
                    out=dbg_x1T[c * P : (c + 1) * P, :],
                    in_=xT[:, c, :, :].rearrange("p a b -> p (a b)"),
                )
            nc.sync.dma_start(out=dbg_whb[:], in_=WHb_full[:])

        # ---------------- pass E: layer-0 backward (+ fused WH1) ---------
        def post_bwd(t, ps, psd):
            dtmp = misc.tile([P, 1], f32, tag="dsrc")
            nc.vector.tensor_scalar_max(dtmp[:], psd[:], 1.0)
            rtmp = misc.tile([P, 1], f32, tag="rsrc")
            nc.vector.reciprocal(rtmp[:], dtmp[:])
            h1 = hrm.tile([P, D], f32, tag="h1")
            nc.vector.scalar_tensor_tensor(
                out=h1[:],
                in0=ps[:],
                scalar=rtmp[:],
                in1=bbbc[:],
                op0=AluOp.mult,
                op1=AluOp.add,
            )
            h2 = hrm.tile([P, D], f32, tag="h2")
            nc.scalar.activation(out=h2[:], in_=h1[:], func=Act.Relu)
            hsb = misc.tile([P, DC, P], f32, tag="hsb")
            for c in range(DC):
                pt = ps_tr.tile([P, P], f32, tag="tr")
                nc.tensor.transpose(
                    out=pt[:], in_=h2[:, c * P : (c + 1) * P], identity=ident[:]
                )
                nc.vector.tensor_copy(out=hsb[:, c, :], in_=pt[:])
            ps2 = ps_acc.tile([P, D], f32, tag="acc", name=f"psw1{t}")
            nc.tensor.matmul(
                out=ps2[:], lhsT=hsb[:, 0, :], rhs=w1t[:, 0, :], start=True, stop=False
            )
            nc.tensor.matmul(
                out=ps2[:], lhsT=hsb[:, 1, :], rhs=w1t[:, 1, :], start=False, stop=True
            )
            whs = whs_pool.tile([P, D], tb, tag="whs")
            nc.scalar.copy(out=whs[:], in_=ps2[:])
            nc.sync.dma_start(out=WH1_loc[t * P : (t + 1) * P, :], in_=whs[:])

        with tc.tile_pool(name="edgesE", bufs=1) as ep:
            bidx = ep.tile([P, NB * 8], i16, name="bidxE")
            bval = ep.tile([P, NB], f32, name="bvalE")
            bdl = ep.tile([P, NB], f32, name="bdlE")
            nc.sync.dma_start(out=bidx[:], in_=be_i16[:])
            nc.sync.dma_start(out=bval[:], in_=be_val[:])
            nc.sync.dma_start(out=bdl[:], in_=be_dl[:])
            gather_pass(bsegs, bidx, bval, bdl, [WHb_full[:]], True, post_bwd)

        nc.gpsimd.collective_compute(
            "AllGather",
            AluOp.bypass,
            replica_groups=rg,
            ins=[WH1_loc[:].opt()],
            outs=[WH1_full[:].opt()],
        )
        if taps:
            nc.sync.dma_start(out=dbg_wh1[:], in_=WH1_full[:])

        # ---------------- pass G: layer-1 forward ------------------------
        def post_fwd1(t, ps, psd):
            h1 = hrm.tile([P, D], f32, tag="h1")
            nc.vector.scalar_tensor_tensor(
                out=h1[:],
                in0=ps[:],
                scalar=degc[:, t : t + 1],
                in1=b1bc[:],
                op0=AluOp.mult,
                op1=AluOp.add,
            )
            h2 = hrm.tile([P, D], f32, tag="h2")
            nc.scalar.activation(out=h2[:], in_=h1[:], func=Act.Relu)
            for c in range(DC):
                pt = ps_tr.tile([P, P], f32, tag="tr")
                nc.tensor.transpose(
                    out=pt[:], in_=h2[:, c * P : (c + 1) * P], identity=ident[:]
                )
                nc.vector.tensor_add(x2T[:, c, t, :], pt[:], xT[:, c, t, :])

        with tc.tile_pool(name="edgesG", bufs=1) as ep:
            fidx2 = ep.tile([P, NF * 8], i16, name="fidxG")
            fval2 = ep.tile([P, NF], f32, name="fvalG")
            fdl2 = ep.tile([P, NF], f32, name="fdlG")
            nc.sync.dma_start(out=fidx2[:], in_=fe_i16[:])
            nc.sync.dma_start(out=fval2[:], in_=fe_val[:])
            nc.sync.dma_start(out=fdl2[:], in_=fe_dl[:])
            gather_pass(
                fsegs, fidx2, fval2, fdl2, [t_[:] for t_ in WH1_t], False, post_fwd1
            )

        # ---------------- BN-2 + output ----------------------------------
        st2_sb = bn_stats_phase(x2T, tgt_sh, "st2_sb")
        nc.sync.dma_start(out=st2_in[:], in_=st2_sb[:])
        nc.gpsimd.collective_compute(
            "AllReduce",
            AluOp.add,
            replica_groups=rg,
            ins=[st2_in[:].opt()],
            outs=[st2_out[:].opt()],
        )
        st2g = misc.tile([P, 2 * DC], f32, tag="stg")
        nc.sync.dma_start(out=st2g[:], in_=st2_out[:])
        A2, B2 = bn_coeffs(st2g, g2f, be2f, n_tgt, "A2", "B2")

        for t in range(NT):
            for c in range(DC):
                nc.vector.scalar_tensor_tensor(
                    out=x2T[:, c, t, :],
                    in0=x2T[:, c, t, :],
                    scalar=A2[:, c : c + 1],
                    in1=B2[:, c : c + 1].to_broadcast([P, P]),
                    op0=AluOp.mult,
                    op1=AluOp.add,
                )
        for c in range(DC):
            nc.sync.dma_start(
                out=outT[c * P : (c + 1) * P, :],
                in_=x2T[:, c, :, :].rearrange("p a b -> p (a b)"),
            )

    nc.compile()
    return nc


# ----------------------------------------------------------------- entry


def _run(inputs, trace=False, tmpdir=None, taps=False):
    from concourse.bass_utils import run_bass_kernel_spmd

    H_src = np.asarray(inputs["H_src"], dtype=np.float32)
    target_emb = np.asarray(inputs["target_emb"], dtype=np.float32)
    W_fwd = np.asarray(inputs["W_fwd"], dtype=np.float32)
    b_fwd = np.asarray(inputs["b_fwd"], dtype=np.float32)
    W_bwd = np.asarray(inputs["W_bwd"], dtype=np.float32)
    b_bwd = np.asarray(inputs["b_bwd"], dtype=np.float32)
    gamma = np.asarray(inputs["gamma"], dtype=np.float32)
    beta = np.asarray(inputs["beta"], dtype=np.float32)
    vals = np.asarray(inputs["vals"], dtype=np.float32)
    rows = np.asarray(inputs["rows"])
    cols = np.asarray(inputs["cols"])

    n_src, D = H_src.shape
    n_tgt = target_emb.shape[0]
    assert D == D_FIXED
    tgt_sh = n_tgt // NCORES
    DC = D // P

    fsegs, f_i, f_v, f_d = _edge_plan(
        rows, cols, vals, n_tgt, n_src, NCORES, split=True
    )
    bsegs, b_i, b_v, b_d = _edge_plan(
        cols, rows, vals, n_src, n_tgt, NCORES, split=False
    )

    nc = _build_program(n_tgt, n_src, fsegs, bsegs, taps=taps)

    HsrcT = np.ascontiguousarray(H_src.T)
    iota = np.ascontiguousarray(np.tile(np.arange(P, dtype=np.float32), (P, 1)))

    def fmaj(v):  # [D] -> [P, DC] feature-major
        return np.ascontiguousarray(v.reshape(DC, P).T)

    in_maps = []
    for c in range(NCORES):
        in_maps.append(
            {
                "HsrcT": HsrcT,
                "W0": W_fwd[0],
                "Wb": W_bwd[0],
                "W1": W_fwd[1],
                "b0": b_fwd[0].reshape(1, D),
                "bb": b_bwd[0].reshape(1, D),
                "b1": b_fwd[1].reshape(1, D),
                "g1T": fmaj(gamma[0]),
                "be1T": fmaj(beta[0]),
                "g2T": fmaj(gamma[1]),
                "be2T": fmaj(beta[1]),
                "iota": iota,
                "emb": np.ascontiguousarray(
                    target_emb[c * tgt_sh : (c + 1) * tgt_sh]
                ),
                "fe_i16": f_i[c],
                "fe_val": f_v[c],
                "fe_dl": f_d[c],
                "be_i16": b_i[c],
                "be_val": b_v[c],
                "be_dl": b_d[c],
            }
        )

    res = run_bass_kernel_spmd(
        nc, in_maps, list(range(NCORES)), trace=trace, tmpdir=tmpdir
    )
    out = np.concatenate(
        [np.asarray(res.results[c]["outT"]).astype(np.float32).T for c in range(NCORES)],
        axis=0,
    )
    return np.ascontiguousarray(out), res


def kernel(**inputs) -> np.ndarray:
    out, _ = _run(inputs)
    return out



# revision 12
# speedup vs baseline: 1.9540x; 1.9540x over previous
"""Bipartite GCN stack (2 layers) on 8 Trainium2 NeuronCores.

Architecture (v2): associativity + partition-aligned aggregation.

  - A @ (H W + b) == (A @ H) W + deg*b: every sparse aggregation runs on the
    RAW table (H_src / H1' / Hs1) and the dense d x d transform is applied
    per-destination afterwards.  No pre-transformed 64MB tables, no
    redundant dense work.
  - Aggregation: destinations are degree-sorted and dealt round-robin into
    128-row tiles (tile g -> core g%8), so every destination owns one SBUF
    partition.  Each gathered "column" holds one edge per destination;
    msg accumulation is a single DVE scalar_tensor_tensor per column
    (acc += gathered * val[p]), and the degree is a free-axis reduce of the
    val matrix.  No selection-matrix matmuls at all.
  - Gathers: dma_gather with 4 SWDGE queues round-robin (the Q7 ucode runs
    on core pair `queue_num`, so 4 queues pipeline ~2.4x).  65536-row
    tables are addressed with SIGNED int16 indices against a base biased by
    +32768 rows (the ucode sign-extends; only TRAILING negative indices are
    trimmed, so the planner guarantees the last slot of every call is
    non-negative via partition-127 edge placement).
  - BN stats via PE (ones^T @ [x | x^2] accumulated across tiles), 2KB
    AllReduce, coefficients broadcast via a DRAM round-trip.
  - Tables H1' (BN'd layer-1 targets) and Hs1 (layer-1 sources) are
    produced in slot order, AllGathered in bf16, and indexed through the
    host-side slot maps.

Host-side work: integer edge planning (sort/permute/pad) only; all FP math
runs on the NeuronCores.
"""

import numpy as np

P = 128
D_FIXED = 256
EPS = 1e-5
NCORES = 8
GBT = 8          # gather batch: columns (x128 rows) per dma_gather call
NSWQ = 4         # SWDGE queues used round-robin

N_TGT = 32768
N_SRC = 65536


# ----------------------------------------------------------------- host plan


def _rank_within_group(sorted_groups):
    """Given a sorted array of group ids, return the rank of each element
    within its group (0,1,2,... per group)."""
    n = len(sorted_groups)
    if n == 0:
        return np.zeros(0, np.int64)
    first = np.r_[True, sorted_groups[1:] != sorted_groups[:-1]]
    starts = np.where(first, np.arange(n), 0)
    np.maximum.accumulate(starts, out=starts)
    return np.arange(n) - starts


def _constrained_positions(k, ncols):
    """Batch-final slot positions (<k) for a partition-127 destination with
    k edges in a tile with ncols columns (batches cut at multiples of 8)."""
    cuts = list(range(GBT - 1, ncols, GBT))
    if (ncols - 1) not in cuts:
        cuts.append(ncols - 1)
    return [q for q in cuts if q < k]


class _SidePlan:
    pass


def _plan_side(dst, n_dst, ncores):
    """Degree-sorted partition-aligned destination layout for one direction.

    Returns a _SidePlan with:
      part:   [n_tiles, 128] destination ids per (global tile, partition)
      ncols:  [nlt] common column count per local tile
      slot:   [n_dst] -> (core*shard + lt*128 + p) table-row of each dst
      e_core/e_lt/e_p: per-edge placement (column assigned later per pass)
    """
    sp = _SidePlan()
    counts = np.bincount(dst, minlength=n_dst)
    order = np.argsort(-counts, kind="stable")
    n_tiles = n_dst // P
    nlt = n_tiles // ncores
    part = order.reshape(n_tiles, P).copy()
    band_max = counts[order].reshape(nlt, ncores * P).max(axis=1)
    ncols = np.maximum(band_max, 1).astype(np.int64)

    sp.counts = counts
    sp.part = part
    sp.ncols = ncols
    sp.n_tiles = n_tiles
    sp.nlt = nlt
    sp.n_dst = n_dst
    return sp


def _finish_side(sp, ncores, good_masks):
    """Pick partition-127 members (trailing-trim guard) and build slot maps.

    good_masks: list of [n_dst] bool arrays, one per biased pass using this
    side's layout (destination d needs >= |constrained| good edges for EVERY
    pass).  Empty list -> no constraint.
    """
    counts, part, ncols = sp.counts, sp.part, sp.ncols
    if good_masks:
        # per-destination good-edge counts per pass
        for g in range(sp.n_tiles):
            lt = g // ncores
            m = int(ncols[lt])
            members = part[g]
            best, best_slack = 127, None
            for j in range(P):
                t = members[j]
                k = int(counts[t])
                ncon = len(_constrained_positions(k, m))
                slack = min(int(gm[t]) - ncon for gm in good_masks)
                if best_slack is None or slack > best_slack:
                    best, best_slack = j, slack
                    if slack >= 2:
                        break
            assert best_slack is not None and best_slack >= 0, (
                f"tile {g}: no viable partition-127 member (slack {best_slack})"
            )
            if best != 127:
                part[g, 127], part[g, best] = part[g, best], part[g, 127]

    slot = np.empty(sp.n_dst, np.int64)
    n_tiles = sp.n_tiles
    g_idx = np.arange(n_tiles)
    core_of_g = g_idx % ncores
    lt_of_g = g_idx // ncores
    shard = sp.n_dst // ncores
    base = core_of_g * shard + lt_of_g * P
    slot[part] = base[:, None] + np.arange(P)[None, :]
    sp.slot = slot
    sp.col_off = np.concatenate([[0], np.cumsum(sp.ncols)])
    sp.tc = int(sp.col_off[-1])
    return sp


def _assign_columns(sp, dst, tbl_idx, vals, ncores, constrain_good=None):
    """Assign each edge to (core, colg, p) and build idx/val arrays.

    tbl_idx: per-edge int16 table index (may be negative for biased tables).
    constrain_good: bool[n_edges] "good" mask; if given, partition-127
    destinations get good edges placed at constrained positions.
    Returns idx16 [ncores][128, TC*8], val [ncores][128, TC] f32.
    """
    slot = sp.slot[dst]
    shard = sp.n_dst // ncores
    core = slot // shard
    rem = slot % shard
    lt = rem // P
    p = rem % P

    # rank of each edge within its destination
    eorder = np.argsort(slot, kind="stable")
    rank = np.empty(len(dst), np.int64)
    rank[eorder] = _rank_within_group(slot[eorder])

    if constrain_good is not None:
        # re-rank edges of partition-127 destinations: good edges first at
        # constrained positions
        p127 = p == 127
        if p127.any():
            sub = np.where(p127)[0]
            sub_slot = slot[sub]
            so = np.argsort(sub_slot, kind="stable")
            sub_s = sub[so]
            ss = sub_slot[so]
            starts = np.r_[0, np.where(ss[1:] != ss[:-1])[0] + 1]
            ends = np.r_[starts[1:], len(ss)]
            for a, b in zip(starts, ends):
                es = sub_s[a:b]
                k = b - a
                m = int(sp.ncols[(ss[a] % shard) // P])
                cons = _constrained_positions(k, m)
                good = constrain_good[es]
                order_pos = np.full(k, -1, np.int64)
                gi = np.where(good)[0]
                bi = np.where(~good)[0]
                assert len(gi) >= len(cons), "p127 guard violated"
                # good edges at constrained positions, rest fill remaining
                used = set()
                for q, e in zip(cons, gi):
                    order_pos[e] = q
                    used.add(q)
                rest = [q for q in range(k) if q not in used]
                pool = [e for e in gi[len(cons):]] + list(bi)
                for q, e in zip(rest, pool):
                    order_pos[e] = q
                rank[es] = order_pos

    colg = sp.col_off[lt] + rank
    tc = sp.tc

    idx16 = []
    valo = []
    for c in range(ncores):
        sel = core == c
        lin = np.zeros((tc, P), np.int16)
        va = np.zeros((tc, P), np.float32)
        lin[colg[sel], p[sel]] = tbl_idx[sel].astype(np.int16)
        va[colg[sel], p[sel]] = vals[sel]
        # idx layout: per batch [c0, c0+bs): [bs*128] -> [bs*8, 16].T, x8 rows
        out16 = np.zeros((16, tc * 8), np.int16)
        for lt_i in range(sp.nlt):
            c0 = int(sp.col_off[lt_i])
            m = int(sp.ncols[lt_i])
            done = 0
            while done < m:
                bs = min(GBT, m - done)
                blk = lin[c0 + done : c0 + done + bs].reshape(bs * P)
                out16[:, (c0 + done) * 8 : (c0 + done + bs) * 8] = blk.reshape(
                    bs * 8, 16
                ).T
                done += bs
        idx16.append(np.ascontiguousarray(np.tile(out16, (8, 1))))
        valo.append(np.ascontiguousarray(va.T))
    return idx16, valo


def _make_plans(rows, cols, vals):
    """Full host plan for all three aggregation passes."""
    fwd = _plan_side(rows, N_TGT, NCORES)
    bwd = _plan_side(cols, N_SRC, NCORES)

    # bwd has no trailing-trim constraint (table indices 0..32767 >= 0)
    _finish_side(bwd, NCORES, [])

    # fwd feeds pass B (idx = cols - 32768) and pass G (idx = hsrow - 32768)
    hsrow = bwd.slot  # source id -> Hs1 table row
    goodB = np.bincount(rows[cols >= N_SRC // 2], minlength=N_TGT)
    goodG = np.bincount(rows[hsrow[cols] >= N_SRC // 2], minlength=N_TGT)
    _finish_side(fwd, NCORES, [goodB, goodG])
    h1row = fwd.slot  # target id -> H1' table row

    idxB, valB = _assign_columns(
        fwd, rows, cols - N_SRC // 2, vals, NCORES,
        constrain_good=cols >= N_SRC // 2,
    )
    gmapped = hsrow[cols]
    idxG, valG = _assign_columns(
        fwd, rows, gmapped - N_SRC // 2, vals, NCORES,
        constrain_good=gmapped >= N_SRC // 2,
    )
    idxE, valE = _assign_columns(bwd, cols, h1row[rows], vals, NCORES)

    return fwd, bwd, idxB, valB, idxE, valE, idxG, valG


# ----------------------------------------------------------------- bass build


def _install_drain_patch():
    """walrus in this env allows only ONE sem-wait per instruction; split
    extra waits onto same-engine carrier instructions."""
    import concourse.mybir as mybir
    import concourse.tile as _tile
    from concourse.vector_clock import ScopedClock

    if getattr(_tile.TileContext, "_drain_split_patched", False):
        return

    def _split_drain_and_barrier(self, tick_clock, wait_clock):
        nc = self.nc
        drain_inst = nc.sync.drain()
        wait_clock.add_sem_waits(
            drain_inst.ins, ScopedClock({None: tick_clock.global_clock})
        )
        si = drain_inst.ins.sync_info
        waits = list(si.on_wait) if si and si.on_wait else []
        if len(waits) > 1:
            si.on_wait = waits[:1]
            drain_inst.ins.sync_info = si
            for i in range(1, len(waits)):
                extra = nc.sync.drain()
                esi = extra.ins.sync_info
                upd = list(esi.on_update) if esi and esi.on_update else []
                extra.ins.sync_info = mybir.SyncInfo(
                    on_wait=[waits[i]], on_update=upd
                )
        nc.all_engine_barrier()
        assert self.sems is not None
        popped = nc._tile_sem_poison_stack.pop()
        assert popped is self._sem_poison
        nc.clear_and_free_semaphores(list(self.sems.allocated().values()))
        nc.all_engine_barrier()

    _tile.TileContext._drain_and_barrier = _split_drain_and_barrier

    _orig_add = _tile.TileContext._add_instruction

    def _add_instruction_split(self, inst):
        si = inst.sync_info
        waits = list(si.on_wait) if si and si.on_wait else []
        if len(waits) > 1 and inst.engine != mybir.EngineType.Unassigned:
            for w in waits[:-1]:
                nop = mybir.InstNoOp(
                    name=self.nc.get_next_instruction_name(), ins=[], outs=[]
                )
                nop.engine = inst.engine
                nop.sync_info = mybir.SyncInfo(on_wait=[w], on_update=[])
                _orig_add(self, nop)
            si.on_wait = waits[-1:]
            inst.sync_info = si
        _orig_add(self, inst)

    _tile.TileContext._add_instruction = _add_instruction_split
    _tile.TileContext._drain_split_patched = True


def _build_program(ncolsF, ncolsB):
    from contextlib import ExitStack

    import concourse.bass as bass
    import concourse.mybir as mybir
    import concourse.tile as tile
    from concourse import bacc
    from concourse.masks import make_identity

    _install_drain_patch()

    dt = mybir.dt
    f32 = dt.float32
    bf16 = dt.bfloat16
    i16 = dt.int16
    D = D_FIXED
    DC = D // P
    NTF = len(ncolsF)   # fwd tiles per core (32)
    NTB = len(ncolsB)   # bwd tiles per core (64)
    TCF = int(sum(ncolsF))
    TCB = int(sum(ncolsB))
    tgt_sh = NTF * P
    src_sh = NTB * P
    AluOp = mybir.AluOpType
    Act = mybir.ActivationFunctionType
    Axis = mybir.AxisListType
    rg = [list(range(NCORES))]

    nc = bacc.Bacc(
        "TRN2", target_bir_lowering=False, debug=False,
        num_devices=NCORES, num_swdge_queues=NSWQ,
    )

    dram_t = nc.dram_tensor
    # gathered with signed idx against a base biased by +32768 rows
    Hsrc = dram_t("Hsrc", [N_SRC, D], f32, kind="ExternalInput").ap()
    emb = dram_t("emb", [tgt_sh, D], f32, kind="ExternalInput").ap()
    W0 = dram_t("W0", [D, D], f32, kind="ExternalInput").ap()
    Wb = dram_t("Wb", [D, D], f32, kind="ExternalInput").ap()
    W1 = dram_t("W1", [D, D], f32, kind="ExternalInput").ap()
    b0_h = dram_t("b0", [1, D], f32, kind="ExternalInput")
    bb_h = dram_t("bb", [1, D], f32, kind="ExternalInput")
    b1_h = dram_t("b1", [1, D], f32, kind="ExternalInput")
    g1_h = dram_t("g1", [1, D], f32, kind="ExternalInput").ap()
    be1_h = dram_t("be1", [1, D], f32, kind="ExternalInput").ap()
    g2_h = dram_t("g2", [1, D], f32, kind="ExternalInput").ap()
    be2_h = dram_t("be2", [1, D], f32, kind="ExternalInput").ap()
    fe_i16 = dram_t("fe_i16", [P, TCF * 8], i16, kind="ExternalInput").ap()
    fe_val = dram_t("fe_val", [P, TCF], f32, kind="ExternalInput").ap()
    be_i16 = dram_t("be_i16", [P, TCB * 8], i16, kind="ExternalInput").ap()
    be_val = dram_t("be_val", [P, TCB], f32, kind="ExternalInput").ap()
    ge_i16 = dram_t("ge_i16", [P, TCF * 8], i16, kind="ExternalInput").ap()
    ge_val = dram_t("ge_val", [P, TCF], f32, kind="ExternalInput").ap()
    out_d = dram_t("out", [tgt_sh, D], f32, kind="ExternalOutput").ap()

    with tile.TileContext(nc) as tc, ExitStack() as ctx:
        dram = ctx.enter_context(tc.tile_pool(name="dram", bufs=1, space="DRAM"))
        H1_loc = dram.tile([tgt_sh, D], bf16)
        H1_full = dram.tile([N_TGT, D], bf16, addr_space="Shared")
        Hs1_loc = dram.tile([src_sh, D], bf16)
        Hs1_full = dram.tile([N_SRC, D], bf16, addr_space="Shared")
        st1_in = dram.tile([1, 2 * D], f32)
        st1_out = dram.tile([1, 2 * D], f32, addr_space="Shared")
        st2_in = dram.tile([1, 2 * D], f32)
        st2_out = dram.tile([1, 2 * D], f32, addr_space="Shared")
        ab1_d = dram.tile([1, 2 * D], f32)
        ab2_d = dram.tile([1, 2 * D], f32)

        # ---------------- constants ------------------------------------
        consts = ctx.enter_context(tc.tile_pool(name="consts", bufs=1))
        w0t = consts.tile([P, DC, D], f32)
        wbt = consts.tile([P, DC, D], f32)
        w1t = consts.tile([P, DC, D], f32)
        for c in range(DC):
            nc.sync.dma_start(out=w0t[:, c, :], in_=W0[c * P : (c + 1) * P, :])
            nc.sync.dma_start(out=wbt[:, c, :], in_=Wb[c * P : (c + 1) * P, :])
            nc.sync.dma_start(out=w1t[:, c, :], in_=W1[c * P : (c + 1) * P, :])
        w0b = consts.tile([P, DC, D], bf16)
        wbb = consts.tile([P, DC, D], bf16)
        w1b = consts.tile([P, DC, D], bf16)
        for c in range(DC):
            nc.vector.tensor_copy(out=w0b[:, c, :], in_=w0t[:, c, :])
            nc.vector.tensor_copy(out=wbb[:, c, :], in_=wbt[:, c, :])
            nc.vector.tensor_copy(out=w1b[:, c, :], in_=w1t[:, c, :])
        b0bc = consts.tile([P, D], f32)
        bbbc = consts.tile([P, D], f32)
        b1bc = consts.tile([P, D], f32)
        for h_, t_ in ((b0_h, b0bc), (bb_h, bbbc), (b1_h, b1bc)):
            nc.gpsimd.dma_start(
                out=t_[:], in_=bass.AP(tensor=h_, offset=0, ap=[[0, P], [1, D]])
            )
        g1r = consts.tile([1, D], f32)
        be1r = consts.tile([1, D], f32)
        g2r = consts.tile([1, D], f32)
        be2r = consts.tile([1, D], f32)
        nc.sync.dma_start(out=g1r[:], in_=g1_h[:])
        nc.sync.dma_start(out=be1r[:], in_=be1_h[:])
        nc.sync.dma_start(out=g2r[:], in_=g2_h[:])
        nc.sync.dma_start(out=be2r[:], in_=be2_h[:])
        ident = consts.tile([P, P], f32)
        make_identity(nc, ident[:])
        onesb = consts.tile([P, 1], bf16)
        nc.vector.memset(onesb[:], 1.0)
        epst = consts.tile([1, 1], f32)
        nc.vector.memset(epst[:], EPS)

        # resident state (x2 reuses x1res: x1 is dead once H1' is written)
        degres = consts.tile([P, NTF], f32)       # reciprocal clamped tgt degree
        x1res = consts.tile([P, NTF, D], bf16)    # layer-1 pre-BN x / layer-2 x
        h1res = consts.tile([P, NTF, D], bf16)    # H1' (BN'd)
        x2res = x1res
        a1bc = consts.tile([P, D], f32)
        b1bc2 = consts.tile([P, D], f32)
        a2bc = consts.tile([P, D], f32)
        b2bc2 = consts.tile([P, D], f32)

        acc_pool = ctx.enter_context(tc.tile_pool(name="acc", bufs=4))
        ps_tr = ctx.enter_context(tc.tile_pool(name="pstr", bufs=2, space="PSUM"))
        ps_x = ctx.enter_context(tc.tile_pool(name="psx", bufs=2, space="PSUM"))
        ps_st = ctx.enter_context(tc.tile_pool(name="psst", bufs=1, space="PSUM"))
        zt_pool = ctx.enter_context(tc.tile_pool(name="zt", bufs=2))
        misc = ctx.enter_context(tc.tile_pool(name="misc", bufs=3))
        emb_pool = ctx.enter_context(tc.tile_pool(name="embp", bufs=2))

        st1x = ps_st.tile([1, D], f32, name="st1x")
        st1q = ps_st.tile([1, D], f32, name="st1q")
        st2x = ps_st.tile([1, D], f32, name="st2x")
        st2q = ps_st.tile([1, D], f32, name="st2q")

        qctr = [0]

        def agg_pass(ncols, idx_t, val_t, table_ap, gdt, g_pool, post):
            """Partition-aligned aggregation: per tile, gather columns and
            accumulate acc[p,:] += gathered[p,:] * val[p,col]."""
            for lt in range(len(ncols)):
                m = int(ncols[lt])
                c0 = int(sum(int(x) for x in ncols[:lt]))
                acc = acc_pool.tile([P, D], f32, tag="acc")
                nc.vector.memset(acc[:], 0.0)
                done = 0
                while done < m:
                    bs = min(GBT, m - done)
                    gt = g_pool.tile([P, GBT, D], gdt, tag="gt")
                    nc.gpsimd.dma_gather(
                        out_ap=gt[:, 0:bs, :],
                        in_ap=table_ap,
                        idxs_ap=idx_t[:, (c0 + done) * 8 : (c0 + done + bs) * 8],
                        num_idxs=bs * P,
                        num_idxs_reg=bs * P,
                        elem_size=D,
                        single_packet=True,
                        queue_num=qctr[0] % NSWQ,
                    )
                    qctr[0] += 1
                    for i in range(bs):
                        col = c0 + done + i
                        nxt = acc_pool.tile([P, D], f32, tag="acc")
                        nc.vector.scalar_tensor_tensor(
                            out=nxt[:],
                            in0=gt[:, i, :],
                            scalar=val_t[:, col : col + 1],
                            in1=acc[:],
                            op0=AluOp.mult,
                            op1=AluOp.add,
                        )
                        acc = nxt
                    done += bs
                # degree = sum of vals along columns
                dsum = misc.tile([P, 1], f32, tag="dsum")
                nc.vector.tensor_reduce(
                    out=dsum[:], in_=val_t[:, c0 : c0 + m], axis=Axis.X,
                    op=AluOp.add,
                )
                dcl = misc.tile([P, 1], f32, tag="dcl")
                nc.vector.tensor_scalar_max(dcl[:], dsum[:], 1.0)
                rd = misc.tile([P, 1], f32, tag="rd")
                nc.vector.reciprocal(rd[:], dcl[:])
                post(lt, acc, rd)

        def transform(z, wchunks):
            """z [P, D] f32 (dst-major) -> PSUM [P, D] f32 = z @ W."""
            zt = zt_pool.tile([P, DC, P], bf16, tag="zt")
            for c in range(DC):
                pt = ps_tr.tile([P, P], f32, tag="tr")
                nc.tensor.transpose(
                    out=pt[:], in_=z[:, c * P : (c + 1) * P], identity=ident[:]
                )
                nc.scalar.copy(out=zt[:, c, :], in_=pt[:])
            ps = ps_x.tile([P, D], f32, tag="psx")
            nc.tensor.matmul(
                out=ps[:], lhsT=zt[:, 0, :], rhs=wchunks[:, 0, :],
                start=True, stop=False,
            )
            nc.tensor.matmul(
                out=ps[:], lhsT=zt[:, 1, :], rhs=wchunks[:, 1, :],
                start=False, stop=True,
            )
            return ps

        # ================= pass B: layer-1 forward =====================
        def post_fwd0(lt, acc, rd):
            nc.vector.tensor_copy(out=degres[:, lt : lt + 1], in_=rd[:])
            z = misc.tile([P, D], f32, tag="z")
            nc.scalar.activation(
                out=z[:], in_=acc[:], func=Act.Copy, scale=rd[:]
            )
            ps = transform(z, w0b)
            et = emb_pool.tile([P, D], f32, tag="emb")
            nc.sync.dma_start(out=et[:], in_=emb[lt * P : (lt + 1) * P, :])
            t1 = misc.tile([P, D], f32, tag="t1")
            nc.vector.tensor_add(t1[:], ps[:], b0bc[:])
            t2 = misc.tile([P, D], f32, tag="t2")
            nc.scalar.activation(out=t2[:], in_=t1[:], func=Act.Relu)
            nc.vector.tensor_add(x1res[:, lt, :], t2[:], et[:])
            sq = misc.tile([P, D], bf16, tag="sq")
            nc.vector.tensor_mul(sq[:], x1res[:, lt, :], x1res[:, lt, :])
            nc.tensor.matmul(
                out=st1x[:], lhsT=onesb[:], rhs=x1res[:, lt, :],
                start=(lt == 0), stop=(lt == NTF - 1),
            )
            nc.tensor.matmul(
                out=st1q[:], lhsT=onesb[:], rhs=sq[:],
                start=(lt == 0), stop=(lt == NTF - 1),
            )

        with tc.tile_pool(name="edgeE", bufs=1) as epe:
            bidx = epe.tile([P, TCB * 8], i16, name="bidx")
            bval = epe.tile([P, TCB], f32, name="bval")
            nc.scalar.dma_start(out=bidx[:], in_=be_i16[:])
            nc.scalar.dma_start(out=bval[:], in_=be_val[:])

            with tc.tile_pool(name="edgeB", bufs=1) as epb, \
                 tc.tile_pool(name="gB", bufs=5) as gB:
                fidx = epb.tile([P, TCF * 8], i16, name="fidx")
                fval = epb.tile([P, TCF], f32, name="fval")
                nc.sync.dma_start(out=fidx[:], in_=fe_i16[:])
                nc.sync.dma_start(out=fval[:], in_=fe_val[:])
                agg_pass(ncolsF, fidx, fval, Hsrc[N_SRC // 2 :, :], f32, gB,
                         post_fwd0)

            # ---------------- BN-1 -------------------------------------
            def bn_block(stx, stq, stin, stout, g_r, be_r, abd, abc, bbc):
                st_sb = misc.tile([1, 2 * D], f32, tag="stsb")
                nc.scalar.copy(out=st_sb[:, 0:D], in_=stx[:])
                nc.scalar.copy(out=st_sb[:, D : 2 * D], in_=stq[:])
                nc.sync.dma_start(out=stin[:], in_=st_sb[:])
                nc.gpsimd.collective_compute(
                    "AllReduce", AluOp.add, replica_groups=rg,
                    ins=[stin[:].opt()], outs=[stout[:].opt()],
                )
                stg = misc.tile([1, 2 * D], f32, tag="stg")
                nc.sync.dma_start(out=stg[:], in_=stout[:])
                mean = misc.tile([1, D], f32, tag="mean")
                nc.vector.tensor_scalar_mul(mean[:], stg[:, 0:D], 1.0 / N_TGT)
                q = misc.tile([1, D], f32, tag="q")
                nc.vector.tensor_scalar_mul(q[:], stg[:, D : 2 * D], 1.0 / N_TGT)
                musq = misc.tile([1, D], f32, tag="musq")
                nc.vector.tensor_mul(musq[:], mean[:], mean[:])
                var = misc.tile([1, D], f32, tag="var")
                nc.vector.tensor_tensor(
                    out=var[:], in0=q[:], in1=musq[:], op=AluOp.subtract
                )
                sd = misc.tile([1, D], f32, tag="sd")
                nc.scalar.activation(out=sd[:], in_=var[:], func=Act.Sqrt,
                                     bias=epst[:])
                rstd = misc.tile([1, D], f32, tag="rstd")
                nc.vector.reciprocal(rstd[:], sd[:])
                ab = misc.tile([1, 2 * D], f32, tag="ab")
                nc.vector.tensor_mul(ab[:, 0:D], g_r[:], rstd[:])
                mA = misc.tile([1, D], f32, tag="mA")
                nc.vector.tensor_mul(mA[:], mean[:], ab[:, 0:D])
                nc.vector.tensor_tensor(
                    out=ab[:, D : 2 * D], in0=be_r[:], in1=mA[:],
                    op=AluOp.subtract,
                )
                nc.sync.dma_start(out=abd[:], in_=ab[:])
                abt = abd.tensor if hasattr(abd, "tensor") else abd
                nc.gpsimd.dma_start(
                    out=abc[:],
                    in_=bass.AP(tensor=abd[:].tensor, offset=abd[:].offset,
                                ap=[[0, P], [1, D]]),
                )
                nc.gpsimd.dma_start(
                    out=bbc[:],
                    in_=bass.AP(tensor=abd[:].tensor, offset=abd[:].offset + D,
                                ap=[[0, P], [1, D]]),
                )

            bn_block(st1x, st1q, st1_in, st1_out, g1r, be1r, ab1_d, a1bc, b1bc2)

            # H1' = A1*x1 + B1, write table rows
            for lt in range(NTF):
                tmp = misc.tile([P, D], f32, tag="h1tmp")
                nc.vector.tensor_mul(tmp[:], x1res[:, lt, :], a1bc[:])
                nc.vector.tensor_add(h1res[:, lt, :], tmp[:], b1bc2[:])
                nc.sync.dma_start(
                    out=H1_loc[lt * P : (lt + 1) * P, :], in_=h1res[:, lt, :]
                )
            nc.gpsimd.collective_compute(
                "AllGather", AluOp.bypass, replica_groups=rg,
                ins=[H1_loc[:].opt()], outs=[H1_full[:].opt()],
            )

            # ================= pass E: layer-1 backward ================
            def post_bwd(lt, acc, rd):
                z = misc.tile([P, D], f32, tag="z")
                nc.scalar.activation(
                    out=z[:], in_=acc[:], func=Act.Copy, scale=rd[:]
                )
                ps = transform(z, wbb)
                t1 = misc.tile([P, D], f32, tag="t1")
                nc.vector.tensor_add(t1[:], ps[:], bbbc[:])
                hs = misc.tile([P, D], bf16, tag="hs")
                nc.scalar.activation(out=hs[:], in_=t1[:], func=Act.Relu)
                nc.sync.dma_start(
                    out=Hs1_loc[lt * P : (lt + 1) * P, :], in_=hs[:]
                )

            with tc.tile_pool(name="edgeG", bufs=1) as epg:
                gidx = epg.tile([P, TCF * 8], i16, name="gidx")
                gval = epg.tile([P, TCF], f32, name="gval")
                nc.scalar.dma_start(out=gidx[:], in_=ge_i16[:])
                nc.scalar.dma_start(out=gval[:], in_=ge_val[:])

                with tc.tile_pool(name="gE", bufs=8) as gE:
                    agg_pass(ncolsB, bidx, bval, H1_full[:], bf16, gE, post_bwd)

                nc.gpsimd.collective_compute(
                    "AllGather", AluOp.bypass, replica_groups=rg,
                    ins=[Hs1_loc[:].opt()], outs=[Hs1_full[:].opt()],
                )

                # ================= pass G: layer-2 forward =============
                def post_fwd1(lt, acc, rd):
                    z = misc.tile([P, D], f32, tag="z")
                    nc.scalar.activation(
                        out=z[:], in_=acc[:], func=Act.Copy,
                        scale=degres[:, lt : lt + 1],
                    )
                    ps = transform(z, w1b)
                    t1 = misc.tile([P, D], f32, tag="t1")
                    nc.vector.tensor_add(t1[:], ps[:], b1bc[:])
                    t2 = misc.tile([P, D], f32, tag="t2")
                    nc.scalar.activation(out=t2[:], in_=t1[:], func=Act.Relu)
                    nc.vector.tensor_add(
                        x2res[:, lt, :], t2[:], h1res[:, lt, :]
                    )
                    sq = misc.tile([P, D], bf16, tag="sq")
                    nc.vector.tensor_mul(
                        sq[:], x2res[:, lt, :], x2res[:, lt, :]
                    )
                    nc.tensor.matmul(
                        out=st2x[:], lhsT=onesb[:], rhs=x2res[:, lt, :],
                        start=(lt == 0), stop=(lt == NTF - 1),
                    )
                    nc.tensor.matmul(
                        out=st2q[:], lhsT=onesb[:], rhs=sq[:],
                        start=(lt == 0), stop=(lt == NTF - 1),
                    )

                def post_fwd1_nodeg(lt, acc, rd):
                    post_fwd1(lt, acc, rd)

                with tc.tile_pool(name="gG", bufs=8) as gG:
                    agg_pass(ncolsF, gidx, gval, Hs1_full[N_SRC // 2 :, :],
                             bf16, gG, post_fwd1)

            # ---------------- BN-2 + output ----------------------------
            bn_block(st2x, st2q, st2_in, st2_out, g2r, be2r, ab2_d, a2bc, b2bc2)
            for lt in range(NTF):
                tmp = misc.tile([P, D], f32, tag="o1")
                nc.vector.tensor_mul(tmp[:], x2res[:, lt, :], a2bc[:])
                ot = misc.tile([P, D], f32, tag="ot")
                nc.vector.tensor_add(ot[:], tmp[:], b2bc2[:])
                nc.sync.dma_start(
                    out=out_d[lt * P : (lt + 1) * P, :], in_=ot[:]
                )

    nc.compile()
    return nc


# ----------------------------------------------------------------- entry


def _run(inputs, trace=False, tmpdir=None):
    from concourse.bass_utils import run_bass_kernel_spmd

    H_src = np.asarray(inputs["H_src"], dtype=np.float32)
    target_emb = np.asarray(inputs["target_emb"], dtype=np.float32)
    W_fwd = np.asarray(inputs["W_fwd"], dtype=np.float32)
    b_fwd = np.asarray(inputs["b_fwd"], dtype=np.float32)
    W_bwd = np.asarray(inputs["W_bwd"], dtype=np.float32)
    b_bwd = np.asarray(inputs["b_bwd"], dtype=np.float32)
    gamma = np.asarray(inputs["gamma"], dtype=np.float32)
    beta = np.asarray(inputs["beta"], dtype=np.float32)
    vals = np.asarray(inputs["vals"], dtype=np.float32)
    rows = np.asarray(inputs["rows"]).astype(np.int64)
    cols = np.asarray(inputs["cols"]).astype(np.int64)

    n_src, D = H_src.shape
    n_tgt = target_emb.shape[0]
    assert D == D_FIXED and n_tgt == N_TGT and n_src == N_SRC

    fwd, bwd, idxB, valB, idxE, valE, idxG, valG = _make_plans(rows, cols, vals)

    ncolsF_core = fwd.ncols
    ncolsB_core = bwd.ncols
    nc = _build_program(list(ncolsF_core), list(ncolsB_core))

    # per-core permuted emb rows / output slots
    part = fwd.part  # [256, 128] target ids (post-swap)
    in_maps = []
    perms = []
    for c in range(NCORES):
        tiles = part[c::NCORES]              # [NTF, 128] lt-major
        perm = tiles.reshape(-1)
        perms.append(perm)
        in_maps.append(
            {
                "Hsrc": H_src,
                "emb": np.ascontiguousarray(target_emb[perm]),
                "W0": W_fwd[0],
                "Wb": W_bwd[0],
                "W1": W_fwd[1],
                "b0": b_fwd[0].reshape(1, D),
                "bb": b_bwd[0].reshape(1, D),
                "b1": b_fwd[1].reshape(1, D),
                "g1": gamma[0].reshape(1, D),
                "be1": beta[0].reshape(1, D),
                "g2": gamma[1].reshape(1, D),
                "be2": beta[1].reshape(1, D),
                "fe_i16": idxB[c],
                "fe_val": valB[c],
                "be_i16": idxE[c],
                "be_val": valE[c],
                "ge_i16": idxG[c],
                "ge_val": valG[c],
            }
        )

    res = run_bass_kernel_spmd(
        nc, in_maps, list(range(NCORES)), trace=trace, tmpdir=tmpdir
    )
    out = np.empty((N_TGT, D), np.float32)
    for c in range(NCORES):
        out[perms[c]] = np.asarray(res.results[c]["out"]).astype(np.float32)
    return out, res


def kernel(**inputs) -> np.ndarray:
    out, _ = _run(inputs)
    return out


# revision 21
# speedup vs baseline: 1.9772x; 1.0118x over previous
"""Bipartite GCN stack (2 layers) on 8 Trainium2 NeuronCores.

Architecture (v2): associativity + partition-aligned aggregation.

  - A @ (H W + b) == (A @ H) W + deg*b: every sparse aggregation runs on the
    RAW table (H_src / H1' / Hs1) and the dense d x d transform is applied
    per-destination afterwards.  No pre-transformed 64MB tables, no
    redundant dense work.
  - Aggregation: destinations are degree-sorted and dealt round-robin into
    128-row tiles (tile g -> core g%8), so every destination owns one SBUF
    partition.  Each gathered "column" holds one edge per destination;
    msg accumulation is a single DVE scalar_tensor_tensor per column
    (acc += gathered * val[p]), and the degree is a free-axis reduce of the
    val matrix.  No selection-matrix matmuls at all.
  - Gathers: dma_gather with 4 SWDGE queues round-robin (the Q7 ucode runs
    on core pair `queue_num`, so 4 queues pipeline ~2.4x).  65536-row
    tables are addressed with SIGNED int16 indices against a base biased by
    +32768 rows (the ucode sign-extends; only TRAILING negative indices are
    trimmed, so the planner guarantees the last slot of every call is
    non-negative via partition-127 edge placement).
  - BN stats via PE (ones^T @ [x | x^2] accumulated across tiles), 2KB
    AllReduce, coefficients broadcast via a DRAM round-trip.
  - Tables H1' (BN'd layer-1 targets) and Hs1 (layer-1 sources) are
    produced in slot order, AllGathered in bf16, and indexed through the
    host-side slot maps.

Host-side work: integer edge planning (sort/permute/pad) only; all FP math
runs on the NeuronCores.
"""

import numpy as np

P = 128
D_FIXED = 256
EPS = 1e-5
NCORES = 8
GBT = 8          # gather batch: columns (x128 rows) per dma_gather call
NSWQ = 4         # SWDGE queues used round-robin

N_TGT = 32768
N_SRC = 65536


# ----------------------------------------------------------------- host plan


def _rank_within_group(sorted_groups):
    """Given a sorted array of group ids, return the rank of each element
    within its group (0,1,2,... per group)."""
    n = len(sorted_groups)
    if n == 0:
        return np.zeros(0, np.int64)
    first = np.r_[True, sorted_groups[1:] != sorted_groups[:-1]]
    starts = np.where(first, np.arange(n), 0)
    np.maximum.accumulate(starts, out=starts)
    return np.arange(n) - starts


def _constrained_positions(k, ncols, col0, tc):
    """Batch-final slot positions (<k) for a partition-127 destination with
    k edges in a tile spanning global columns [col0, col0+ncols) (batches cut
    at GLOBAL column multiples of 8, plus the very last column tc-1)."""
    cuts = [q - col0 for q in range(GBT - 1, col0 + ncols, GBT)
            if q >= col0]
    last = tc - 1 - col0
    if 0 <= last < ncols and last not in cuts:
        cuts.append(last)
    return [q for q in cuts if q < k]


class _SidePlan:
    pass


def _plan_side(dst, n_dst, ncores):
    """Degree-sorted partition-aligned destination layout for one direction.

    Returns a _SidePlan with:
      part:   [n_tiles, 128] destination ids per (global tile, partition)
      ncols:  [nlt] common column count per local tile
      slot:   [n_dst] -> (core*shard + lt*128 + p) table-row of each dst
      e_core/e_lt/e_p: per-edge placement (column assigned later per pass)
    """
    sp = _SidePlan()
    counts = np.bincount(dst, minlength=n_dst)
    order = np.argsort(-counts, kind="stable")
    n_tiles = n_dst // P
    nlt = n_tiles // ncores
    part = order.reshape(n_tiles, P).copy()
    band_max = counts[order].reshape(nlt, ncores * P).max(axis=1)
    ncols = np.maximum(band_max, 1).astype(np.int64)

    sp.counts = counts
    sp.part = part
    sp.ncols = ncols
    sp.n_tiles = n_tiles
    sp.nlt = nlt
    sp.n_dst = n_dst
    return sp


def _finish_side(sp, ncores, good_masks):
    """Pick partition-127 members (trailing-trim guard) and build slot maps.

    good_masks: list of [n_dst] bool arrays, one per biased pass using this
    side's layout (destination d needs >= |constrained| good edges for EVERY
    pass).  Empty list -> no constraint.
    """
    counts, part, ncols = sp.counts, sp.part, sp.ncols
    col_off = np.concatenate([[0], np.cumsum(ncols)])
    tc = int(col_off[-1])
    if good_masks:
        # per-destination good-edge counts per pass
        for g in range(sp.n_tiles):
            lt = g // ncores
            m = int(ncols[lt])
            c0 = int(col_off[lt])
            members = part[g]
            best, best_slack = 127, None
            for j in range(P):
                t = members[j]
                k = int(counts[t])
                ncon = len(_constrained_positions(k, m, c0, tc))
                slack = min(int(gm[t]) - ncon for gm in good_masks)
                if best_slack is None or slack > best_slack:
                    best, best_slack = j, slack
                    if slack >= 2:
                        break
            assert best_slack is not None and best_slack >= 0, (
                f"tile {g}: no viable partition-127 member (slack {best_slack})"
            )
            if best != 127:
                part[g, 127], part[g, best] = part[g, best], part[g, 127]

    slot = np.empty(sp.n_dst, np.int64)
    n_tiles = sp.n_tiles
    g_idx = np.arange(n_tiles)
    core_of_g = g_idx % ncores
    lt_of_g = g_idx // ncores
    shard = sp.n_dst // ncores
    base = core_of_g * shard + lt_of_g * P
    slot[part] = base[:, None] + np.arange(P)[None, :]
    sp.slot = slot
    sp.col_off = col_off
    sp.tc = tc
    return sp


def _assign_columns(sp, dst, tbl_idx, vals, ncores, constrain_good=None):
    """Assign each edge to (core, colg, p) and build idx/val arrays.

    tbl_idx: per-edge int16 table index (may be negative for biased tables).
    constrain_good: bool[n_edges] "good" mask; if given, partition-127
    destinations get good edges placed at constrained positions.
    Returns idx16 [ncores][128, TC*8], val [ncores][128, TC] f32.
    """
    slot = sp.slot[dst]
    shard = sp.n_dst // ncores
    core = slot // shard
    rem = slot % shard
    lt = rem // P
    p = rem % P

    # rank of each edge within its destination
    eorder = np.argsort(slot, kind="stable")
    rank = np.empty(len(dst), np.int64)
    rank[eorder] = _rank_within_group(slot[eorder])

    if constrain_good is not None:
        # re-rank edges of partition-127 destinations: good edges first at
        # constrained positions
        p127 = p == 127
        if p127.any():
            sub = np.where(p127)[0]
            sub_slot = slot[sub]
            so = np.argsort(sub_slot, kind="stable")
            sub_s = sub[so]
            ss = sub_slot[so]
            starts = np.r_[0, np.where(ss[1:] != ss[:-1])[0] + 1]
            ends = np.r_[starts[1:], len(ss)]
            for a, b in zip(starts, ends):
                es = sub_s[a:b]
                k = b - a
                lt_i = (ss[a] % shard) // P
                m = int(sp.ncols[lt_i])
                cons = _constrained_positions(
                    k, m, int(sp.col_off[lt_i]), sp.tc
                )
                good = constrain_good[es]
                order_pos = np.full(k, -1, np.int64)
                gi = np.where(good)[0]
                bi = np.where(~good)[0]
                assert len(gi) >= len(cons), "p127 guard violated"
                # good edges at constrained positions, rest fill remaining
                used = set()
                for q, e in zip(cons, gi):
                    order_pos[e] = q
                    used.add(q)
                rest = [q for q in range(k) if q not in used]
                pool = [e for e in gi[len(cons):]] + list(bi)
                for q, e in zip(rest, pool):
                    order_pos[e] = q
                rank[es] = order_pos

    colg = sp.col_off[lt] + rank
    tc = sp.tc

    idx16 = []
    valo = []
    for c in range(ncores):
        sel = core == c
        lin = np.zeros((tc, P), np.int16)
        va = np.zeros((tc, P), np.float32)
        lin[colg[sel], p[sel]] = tbl_idx[sel].astype(np.int16)
        va[colg[sel], p[sel]] = vals[sel]
        # idx layout: per GLOBAL batch [b0, b0+bs): [bs*128] -> [bs*8, 16].T
        out16 = np.zeros((16, tc * 8), np.int16)
        b0 = 0
        while b0 < tc:
            bs = min(GBT, tc - b0)
            blk = lin[b0 : b0 + bs].reshape(bs * P)
            out16[:, b0 * 8 : (b0 + bs) * 8] = blk.reshape(bs * 8, 16).T
            b0 += bs
        idx16.append(np.ascontiguousarray(np.tile(out16, (8, 1))))
        valo.append(np.ascontiguousarray(va.T))
    return idx16, valo


def _make_plans(rows, cols, vals):
    """Full host plan for all three aggregation passes."""
    fwd = _plan_side(rows, N_TGT, NCORES)
    bwd = _plan_side(cols, N_SRC, NCORES)

    # bwd has no trailing-trim constraint (table indices 0..32767 >= 0)
    _finish_side(bwd, NCORES, [])

    # fwd feeds pass B (idx = cols - 32768) and pass G (idx = hsrow - 32768)
    hsrow = bwd.slot  # source id -> Hs1 table row
    goodB = np.bincount(rows[cols >= N_SRC // 2], minlength=N_TGT)
    goodG = np.bincount(rows[hsrow[cols] >= N_SRC // 2], minlength=N_TGT)
    _finish_side(fwd, NCORES, [goodB, goodG])
    h1row = fwd.slot  # target id -> H1' table row

    idxB, valB = _assign_columns(
        fwd, rows, cols - N_SRC // 2, vals, NCORES,
        constrain_good=cols >= N_SRC // 2,
    )
    gmapped = hsrow[cols]
    idxG, valG = _assign_columns(
        fwd, rows, gmapped - N_SRC // 2, vals, NCORES,
        constrain_good=gmapped >= N_SRC // 2,
    )
    idxE, valE = _assign_columns(bwd, cols, h1row[rows], vals, NCORES)

    return fwd, bwd, idxB, valB, idxE, valE, idxG, valG


# ----------------------------------------------------------------- bass build


def _install_drain_patch():
    """walrus in this env allows only ONE sem-wait per instruction; split
    extra waits onto same-engine carrier instructions."""
    import concourse.mybir as mybir
    import concourse.tile as _tile
    from concourse.vector_clock import ScopedClock

    if getattr(_tile.TileContext, "_drain_split_patched", False):
        return

    def _split_drain_and_barrier(self, tick_clock, wait_clock):
        nc = self.nc
        drain_inst = nc.sync.drain()
        wait_clock.add_sem_waits(
            drain_inst.ins, ScopedClock({None: tick_clock.global_clock})
        )
        si = drain_inst.ins.sync_info
        waits = list(si.on_wait) if si and si.on_wait else []
        if len(waits) > 1:
            si.on_wait = waits[:1]
            drain_inst.ins.sync_info = si
            for i in range(1, len(waits)):
                extra = nc.sync.drain()
                esi = extra.ins.sync_info
                upd = list(esi.on_update) if esi and esi.on_update else []
                extra.ins.sync_info = mybir.SyncInfo(
                    on_wait=[waits[i]], on_update=upd
                )
        nc.all_engine_barrier()
        assert self.sems is not None
        popped = nc._tile_sem_poison_stack.pop()
        assert popped is self._sem_poison
        nc.clear_and_free_semaphores(list(self.sems.allocated().values()))
        nc.all_engine_barrier()

    _tile.TileContext._drain_and_barrier = _split_drain_and_barrier

    _orig_add = _tile.TileContext._add_instruction

    def _add_instruction_split(self, inst):
        si = inst.sync_info
        waits = list(si.on_wait) if si and si.on_wait else []
        if len(waits) > 1 and inst.engine != mybir.EngineType.Unassigned:
            for w in waits[:-1]:
                nop = mybir.InstNoOp(
                    name=self.nc.get_next_instruction_name(), ins=[], outs=[]
                )
                nop.engine = inst.engine
                nop.sync_info = mybir.SyncInfo(on_wait=[w], on_update=[])
                _orig_add(self, nop)
            si.on_wait = waits[-1:]
            inst.sync_info = si
        _orig_add(self, inst)

    _tile.TileContext._add_instruction = _add_instruction_split
    _tile.TileContext._drain_split_patched = True


def _build_program(ncolsF, ncolsB):
    from contextlib import ExitStack

    import concourse.bass as bass
    import concourse.mybir as mybir
    import concourse.tile as tile
    from concourse import bacc
    from concourse.masks import make_identity

    _install_drain_patch()

    dt = mybir.dt
    f32 = dt.float32
    bf16 = dt.bfloat16
    i16 = dt.int16
    D = D_FIXED
    DC = D // P
    NTF = len(ncolsF)   # fwd tiles per core (32)
    NTB = len(ncolsB)   # bwd tiles per core (64)
    TCF = int(sum(ncolsF))
    TCB = int(sum(ncolsB))
    tgt_sh = NTF * P
    src_sh = NTB * P
    AluOp = mybir.AluOpType
    Act = mybir.ActivationFunctionType
    Axis = mybir.AxisListType
    rg = [list(range(NCORES))]

    nc = bacc.Bacc(
        "TRN2", target_bir_lowering=False, debug=False,
        num_devices=NCORES, num_swdge_queues=NSWQ,
    )

    dram_t = nc.dram_tensor
    # gathered with signed idx against a base biased by +32768 rows
    Hsrc = dram_t("Hsrc", [N_SRC, D], f32, kind="ExternalInput").ap()
    emb = dram_t("emb", [tgt_sh, D], f32, kind="ExternalInput").ap()
    W0 = dram_t("W0", [D, D], f32, kind="ExternalInput").ap()
    Wb = dram_t("Wb", [D, D], f32, kind="ExternalInput").ap()
    W1 = dram_t("W1", [D, D], f32, kind="ExternalInput").ap()
    b0_h = dram_t("b0", [1, D], f32, kind="ExternalInput")
    bb_h = dram_t("bb", [1, D], f32, kind="ExternalInput")
    b1_h = dram_t("b1", [1, D], f32, kind="ExternalInput")
    g1_h = dram_t("g1", [1, D], f32, kind="ExternalInput").ap()
    be1_h = dram_t("be1", [1, D], f32, kind="ExternalInput").ap()
    g2_h = dram_t("g2", [1, D], f32, kind="ExternalInput").ap()
    be2_h = dram_t("be2", [1, D], f32, kind="ExternalInput").ap()
    fe_i16 = dram_t("fe_i16", [P, TCF * 8], i16, kind="ExternalInput").ap()
    fe_val = dram_t("fe_val", [P, TCF], f32, kind="ExternalInput").ap()
    be_i16 = dram_t("be_i16", [P, TCB * 8], i16, kind="ExternalInput").ap()
    be_val = dram_t("be_val", [P, TCB], f32, kind="ExternalInput").ap()
    ge_i16 = dram_t("ge_i16", [P, TCF * 8], i16, kind="ExternalInput").ap()
    ge_val = dram_t("ge_val", [P, TCF], f32, kind="ExternalInput").ap()
    out_d = dram_t("out", [tgt_sh, D], f32, kind="ExternalOutput").ap()

    with tile.TileContext(nc) as tc, ExitStack() as ctx:
        dram = ctx.enter_context(tc.tile_pool(name="dram", bufs=1, space="DRAM"))
        X1_loc = dram.tile([tgt_sh, D], bf16)
        X1_full = dram.tile([N_TGT, D], bf16, addr_space="Shared")
        Hs1_loc = dram.tile([src_sh, D], bf16)
        Hs1_full = dram.tile([N_SRC, D], bf16, addr_space="Shared")
        st1_in = dram.tile([1, 2 * D], f32)
        st1_out = dram.tile([1, 2 * D], f32, addr_space="Shared")
        st2_in = dram.tile([1, 2 * D], f32)
        st2_out = dram.tile([1, 2 * D], f32, addr_space="Shared")
        ab1_d = dram.tile([1, 2 * D], f32)
        ab2_d = dram.tile([1, 2 * D], f32)

        # ---------------- constants ------------------------------------
        consts = ctx.enter_context(tc.tile_pool(name="consts", bufs=1))
        w0t = consts.tile([P, DC, D], f32)
        wbt = consts.tile([P, DC, D], f32)
        w1t = consts.tile([P, DC, D], f32)
        for c in range(DC):
            nc.sync.dma_start(out=w0t[:, c, :], in_=W0[c * P : (c + 1) * P, :])
            nc.sync.dma_start(out=wbt[:, c, :], in_=Wb[c * P : (c + 1) * P, :])
            nc.sync.dma_start(out=w1t[:, c, :], in_=W1[c * P : (c + 1) * P, :])
        w0b = consts.tile([P, DC, D], bf16)
        wbb = consts.tile([P, DC, D], bf16)
        w1b = consts.tile([P, DC, D], bf16)
        for c in range(DC):
            nc.vector.tensor_copy(out=w0b[:, c, :], in_=w0t[:, c, :])
            nc.vector.tensor_copy(out=wbb[:, c, :], in_=wbt[:, c, :])
            nc.vector.tensor_copy(out=w1b[:, c, :], in_=w1t[:, c, :])
        b0bc = consts.tile([P, D], f32)
        bbbc = consts.tile([P, D], f32)
        b1bc = consts.tile([P, D], f32)
        for h_, t_ in ((b0_h, b0bc), (bb_h, bbbc), (b1_h, b1bc)):
            nc.gpsimd.dma_start(
                out=t_[:], in_=bass.AP(tensor=h_, offset=0, ap=[[0, P], [1, D]])
            )
        g1r = consts.tile([1, D], f32)
        be1r = consts.tile([1, D], f32)
        g2r = consts.tile([1, D], f32)
        be2r = consts.tile([1, D], f32)
        nc.sync.dma_start(out=g1r[:], in_=g1_h[:])
        nc.sync.dma_start(out=be1r[:], in_=be1_h[:])
        nc.sync.dma_start(out=g2r[:], in_=g2_h[:])
        nc.sync.dma_start(out=be2r[:], in_=be2_h[:])
        ident = consts.tile([P, P], f32)
        make_identity(nc, ident[:])
        onesb = consts.tile([P, 1], bf16)
        nc.vector.memset(onesb[:], 1.0)
        epst = consts.tile([1, 1], f32)
        nc.vector.memset(epst[:], EPS)

        # resident state (x2 reuses x1res: x1 is dead once H1' is written)
        degres = consts.tile([P, NTF], f32)       # reciprocal clamped tgt degree
        x1res = consts.tile([P, NTF, D], bf16)    # layer-1 pre-BN x / layer-2 x
        h1res = consts.tile([P, NTF, D], bf16)    # H1' (BN'd)
        x2res = x1res
        a1bc = consts.tile([P, D], f32)
        b1bc2 = consts.tile([P, D], f32)
        a2bc = consts.tile([P, D], f32)
        b2bc2 = consts.tile([P, D], f32)

        acc_pool = ctx.enter_context(tc.tile_pool(name="acc", bufs=4))
        ps_tr = ctx.enter_context(tc.tile_pool(name="pstr", bufs=2, space="PSUM"))
        ps_x = ctx.enter_context(tc.tile_pool(name="psx", bufs=2, space="PSUM"))
        ps_st = ctx.enter_context(tc.tile_pool(name="psst", bufs=1, space="PSUM"))
        zt_pool = ctx.enter_context(tc.tile_pool(name="zt", bufs=2))
        misc = ctx.enter_context(tc.tile_pool(name="misc", bufs=3))
        emb_pool = ctx.enter_context(tc.tile_pool(name="embp", bufs=2))

        st1 = ps_st.tile([1, 2 * D], f32, name="st1")
        st2 = ps_st.tile([1, 2 * D], f32, name="st2")
        st1x, st1q = st1[:, 0:D], st1[:, D : 2 * D]
        st2x, st2q = st2[:, 0:D], st2[:, D : 2 * D]

        qctr = [0]

        def agg_pass(ncols, idx_t, val_t, table_ap, gdt, g_pool, post):
            """Partition-aligned aggregation, flat cross-tile gather batches:
            acc[p,:] += gathered[p,:] * val[p,col]; per-tile deg + post."""
            mlist = [int(x) for x in ncols]
            tcn = sum(mlist)
            bounds = []
            off = 0
            for m in mlist:
                bounds.append((off, off + m))
                off += m
            tile_of = np.repeat(np.arange(len(mlist)), mlist)
            acc = None
            b0 = 0
            while b0 < tcn:
                bs = min(GBT, tcn - b0)
                gt = g_pool.tile([P, GBT, D], gdt, tag="gt")
                nc.gpsimd.dma_gather(
                    out_ap=gt[:, 0:bs, :],
                    in_ap=table_ap,
                    idxs_ap=idx_t[:, b0 * 8 : (b0 + bs) * 8],
                    num_idxs=bs * P,
                    num_idxs_reg=bs * P,
                    elem_size=D,
                    single_packet=True,
                    queue_num=qctr[0] % NSWQ,
                )
                qctr[0] += 1
                for i in range(bs):
                    col = b0 + i
                    lt = int(tile_of[col])
                    a0, a1_ = bounds[lt]
                    if col == a0:
                        acc = acc_pool.tile([P, D], f32, tag="acc")
                        nc.vector.memset(acc[:], 0.0)
                    nxt = acc_pool.tile([P, D], f32, tag="acc")
                    nc.vector.scalar_tensor_tensor(
                        out=nxt[:],
                        in0=gt[:, i, :],
                        scalar=val_t[:, col : col + 1],
                        in1=acc[:],
                        op0=AluOp.mult,
                        op1=AluOp.add,
                    )
                    acc = nxt
                    if col == a1_ - 1:
                        dsum = misc.tile([P, 1], f32, tag="dsum")
                        nc.vector.tensor_reduce(
                            out=dsum[:], in_=val_t[:, a0:a1_], axis=Axis.X,
                            op=AluOp.add,
                        )
                        dcl = misc.tile([P, 1], f32, tag="dcl")
                        nc.vector.tensor_scalar_max(dcl[:], dsum[:], 1.0)
                        rd = misc.tile([P, 1], f32, tag="rd")
                        nc.vector.reciprocal(rd[:], dcl[:])
                        post(lt, acc, rd)
                b0 += bs

        def transform(z, wchunks):
            """z [P, D] f32 (dst-major) -> PSUM [P, D] f32 = z @ W."""
            zt = zt_pool.tile([P, DC, P], bf16, tag="zt")
            for c in range(DC):
                pt = ps_tr.tile([P, P], f32, tag="tr")
                nc.tensor.transpose(
                    out=pt[:], in_=z[:, c * P : (c + 1) * P], identity=ident[:]
                )
                nc.scalar.copy(out=zt[:, c, :], in_=pt[:])
            ps = ps_x.tile([P, D], f32, tag="psx")
            nc.tensor.matmul(
                out=ps[:], lhsT=zt[:, 0, :], rhs=wchunks[:, 0, :],
                start=True, stop=False,
            )
            nc.tensor.matmul(
                out=ps[:], lhsT=zt[:, 1, :], rhs=wchunks[:, 1, :],
                start=False, stop=True,
            )
            return ps

        ones1f = consts.tile([1, P], f32)
        nc.vector.memset(ones1f[:], 1.0)
        ps_bn = ctx.enter_context(tc.tile_pool(name="psbn", bufs=1, space="PSUM"))

        def bn_ar(stx, stq, stin, stout):
            """Pack stats and launch the AllReduce."""
            st_sb = misc.tile([1, 2 * D], f32, tag="stsb")
            nc.scalar.copy(out=st_sb[:, 0:D], in_=stx)
            nc.scalar.copy(out=st_sb[:, D : 2 * D], in_=stq)
            nc.sync.dma_start(out=stin[:], in_=st_sb[:])
            nc.gpsimd.collective_compute(
                "AllReduce", AluOp.add, replica_groups=rg,
                ins=[stin[:].opt()], outs=[stout[:].opt()],
            )

        def bn_cf(stout, g_r, be_r, abc, bbc):
            """Coefficients A=gamma/std, B=beta-mean*A, broadcast via PE."""
            stg = misc.tile([1, 2 * D], f32, tag="stg")
            nc.sync.dma_start(out=stg[:], in_=stout[:])
            mean = misc.tile([1, D], f32, tag="mean")
            nc.vector.tensor_scalar_mul(mean[:], stg[:, 0:D], 1.0 / N_TGT)
            q = misc.tile([1, D], f32, tag="q")
            nc.vector.tensor_scalar_mul(q[:], stg[:, D : 2 * D], 1.0 / N_TGT)
            musq = misc.tile([1, D], f32, tag="musq")
            nc.vector.tensor_mul(musq[:], mean[:], mean[:])
            var = misc.tile([1, D], f32, tag="var")
            nc.vector.tensor_tensor(
                out=var[:], in0=q[:], in1=musq[:], op=AluOp.subtract
            )
            sd = misc.tile([1, D], f32, tag="sd")
            nc.scalar.activation(out=sd[:], in_=var[:], func=Act.Sqrt,
                                 bias=epst[:])
            rstd = misc.tile([1, D], f32, tag="rstd")
            nc.vector.reciprocal(rstd[:], sd[:])
            ab = misc.tile([1, 2 * D], f32, tag="ab")
            nc.vector.tensor_mul(ab[:, 0:D], g_r[:], rstd[:])
            mA = misc.tile([1, D], f32, tag="mA")
            nc.vector.tensor_mul(mA[:], mean[:], ab[:, 0:D])
            nc.vector.tensor_tensor(
                out=ab[:, D : 2 * D], in0=be_r[:], in1=mA[:],
                op=AluOp.subtract,
            )
            psab = ps_bn.tile([P, 2 * D], f32, tag="psab")
            nc.tensor.matmul(out=psab[:], lhsT=ones1f[:], rhs=ab[:],
                             start=True, stop=True)
            nc.scalar.copy(out=abc[:], in_=psab[:, 0:D])
            nc.scalar.copy(out=bbc[:], in_=psab[:, D : 2 * D])

        # ================= pass B: layer-1 forward =====================
        def post_fwd0(lt, acc, rd):
            nc.vector.tensor_copy(out=degres[:, lt : lt + 1], in_=rd[:])
            z = misc.tile([P, D], f32, tag="z")
            nc.scalar.activation(
                out=z[:], in_=acc[:], func=Act.Copy, scale=rd[:]
            )
            ps = transform(z, w0b)
            et = emb_pool.tile([P, D], f32, tag="emb")
            nc.sync.dma_start(out=et[:], in_=emb[lt * P : (lt + 1) * P, :])
            t1 = misc.tile([P, D], f32, tag="t1")
            nc.vector.tensor_add(t1[:], ps[:], b0bc[:])
            t2 = misc.tile([P, D], f32, tag="t2")
            nc.scalar.activation(out=t2[:], in_=t1[:], func=Act.Relu)
            nc.vector.tensor_add(x1res[:, lt, :], t2[:], et[:])
            nc.sync.dma_start(
                out=X1_loc[lt * P : (lt + 1) * P, :], in_=x1res[:, lt, :]
            )
            sq = misc.tile([P, D], bf16, tag="sq")
            nc.vector.tensor_mul(sq[:], x1res[:, lt, :], x1res[:, lt, :])
            nc.tensor.matmul(
                out=st1x, lhsT=onesb[:], rhs=x1res[:, lt, :],
                start=(lt == 0), stop=(lt == NTF - 1),
            )
            nc.tensor.matmul(
                out=st1q, lhsT=onesb[:], rhs=sq[:],
                start=(lt == 0), stop=(lt == NTF - 1),
            )

        with tc.tile_pool(name="edgeE", bufs=1) as epe:
            bidx = epe.tile([P, TCB * 8], i16, name="bidx")
            bval = epe.tile([P, TCB], f32, name="bval")
            nc.scalar.dma_start(out=bidx[:], in_=be_i16[:])
            nc.scalar.dma_start(out=bval[:], in_=be_val[:])

            with tc.tile_pool(name="edgeB", bufs=1) as epb, \
                 tc.tile_pool(name="gB", bufs=5) as gB:
                fidx = epb.tile([P, TCF * 8], i16, name="fidx")
                fval = epb.tile([P, TCF], f32, name="fval")
                nc.sync.dma_start(out=fidx[:], in_=fe_i16[:])
                nc.sync.dma_start(out=fval[:], in_=fe_val[:])
                agg_pass(ncolsF, fidx, fval, Hsrc[N_SRC // 2 :, :], f32, gB,
                         post_fwd0)

            # -------- BN-1 (commuted): AllReduce, then AllGather raw x1;
            # coefficient work and H1' apply overlap the AllGather ---------
            bn_ar(st1x, st1q, st1_in, st1_out)
            nc.gpsimd.collective_compute(
                "AllGather", AluOp.bypass, replica_groups=rg,
                ins=[X1_loc[:].opt()], outs=[X1_full[:].opt()],
            )
            bn_cf(st1_out, g1r, be1r, a1bc, b1bc2)

            # f-major A1/B1 via PE transpose (first column of the transpose)
            a1f = consts.tile([P, DC], f32)
            b1f = consts.tile([P, DC], f32)
            for c in range(DC):
                pt = ps_tr.tile([P, P], f32, tag="tr")
                nc.tensor.transpose(out=pt[:], in_=a1bc[:, c * P : (c + 1) * P],
                                    identity=ident[:])
                nc.vector.tensor_copy(out=a1f[:, c : c + 1], in_=pt[:, 0:1])
                pt2 = ps_tr.tile([P, P], f32, tag="tr")
                nc.tensor.transpose(out=pt2[:], in_=b1bc2[:, c * P : (c + 1) * P],
                                    identity=ident[:])
                nc.vector.tensor_copy(out=b1f[:, c : c + 1], in_=pt2[:, 0:1])
            # WbA = diag(A1) @ Wb ; ebias = B1 @ Wb + bb (all per-feature)
            wbA = consts.tile([P, DC, D], bf16)
            for c in range(DC):
                nc.vector.tensor_scalar_mul(
                    wbA[:, c, :], wbt[:, c, :], a1f[:, c : c + 1]
                )
            psbw = ps_x.tile([P, D], f32, tag="psx")
            nc.tensor.matmul(out=psbw[0:1, :], lhsT=b1f[:, 0:1],
                             rhs=wbt[:, 0, :], start=True, stop=False)
            nc.tensor.matmul(out=psbw[0:1, :], lhsT=b1f[:, 1:2],
                             rhs=wbt[:, 1, :], start=False, stop=True)
            bwr = misc.tile([1, D], f32, tag="bwr")
            nc.scalar.copy(out=bwr[:], in_=psbw[0:1, :])
            psbb = ps_x.tile([P, D], f32, tag="psx")
            nc.tensor.matmul(out=psbb[:], lhsT=ones1f[:], rhs=bwr[:],
                             start=True, stop=True)
            ebias = consts.tile([P, D], f32)
            nc.vector.tensor_add(ebias[:], psbb[:], bbbc[:])
            # H1' tiles for the layer-2 residual (overlaps the AllGather)
            for lt in range(NTF):
                tmp = misc.tile([P, D], f32, tag="h1tmp")
                nc.vector.tensor_mul(tmp[:], x1res[:, lt, :], a1bc[:])
                nc.vector.tensor_add(h1res[:, lt, :], tmp[:], b1bc2[:])

            # ================= pass E: layer-1 backward ================
            def post_bwd(lt, acc, rd):
                z = misc.tile([P, D], f32, tag="z")
                nc.scalar.activation(
                    out=z[:], in_=acc[:], func=Act.Copy, scale=rd[:]
                )
                ps = transform(z, wbA)
                t1 = misc.tile([P, D], f32, tag="t1")
                nc.vector.tensor_add(t1[:], ps[:], ebias[:])
                hs = misc.tile([P, D], bf16, tag="hs")
                nc.scalar.activation(out=hs[:], in_=t1[:], func=Act.Relu)
                nc.sync.dma_start(
                    out=Hs1_loc[lt * P : (lt + 1) * P, :], in_=hs[:]
                )

            with tc.tile_pool(name="edgeG", bufs=1) as epg:
                gidx = epg.tile([P, TCF * 8], i16, name="gidx")
                gval = epg.tile([P, TCF], f32, name="gval")
                nc.scalar.dma_start(out=gidx[:], in_=ge_i16[:])
                nc.scalar.dma_start(out=gval[:], in_=ge_val[:])

                with tc.tile_pool(name="gE", bufs=8) as gE:
                    agg_pass(ncolsB, bidx, bval, X1_full[:], bf16, gE, post_bwd)

                nc.gpsimd.collective_compute(
                    "AllGather", AluOp.bypass, replica_groups=rg,
                    ins=[Hs1_loc[:].opt()], outs=[Hs1_full[:].opt()],
                )

                # ================= pass G: layer-2 forward =============
                def post_fwd1(lt, acc, rd):
                    z = misc.tile([P, D], f32, tag="z")
                    nc.scalar.activation(
                        out=z[:], in_=acc[:], func=Act.Copy,
                        scale=degres[:, lt : lt + 1],
                    )
                    ps = transform(z, w1b)
                    t1 = misc.tile([P, D], f32, tag="t1")
                    nc.vector.tensor_add(t1[:], ps[:], b1bc[:])
                    t2 = misc.tile([P, D], f32, tag="t2")
                    nc.scalar.activation(out=t2[:], in_=t1[:], func=Act.Relu)
                    nc.vector.tensor_add(
                        x2res[:, lt, :], t2[:], h1res[:, lt, :]
                    )
                    sq = misc.tile([P, D], bf16, tag="sq")
                    nc.vector.tensor_mul(
                        sq[:], x2res[:, lt, :], x2res[:, lt, :]
                    )
                    nc.tensor.matmul(
                        out=st2x, lhsT=onesb[:], rhs=x2res[:, lt, :],
                        start=(lt == 0), stop=(lt == NTF - 1),
                    )
                    nc.tensor.matmul(
                        out=st2q, lhsT=onesb[:], rhs=sq[:],
                        start=(lt == 0), stop=(lt == NTF - 1),
                    )

                with tc.tile_pool(name="gG", bufs=8) as gG:
                    agg_pass(ncolsF, gidx, gval, Hs1_full[N_SRC // 2 :, :],
                             bf16, gG, post_fwd1)

            # ---------------- BN-2 + output ----------------------------
            bn_ar(st2x, st2q, st2_in, st2_out)
            bn_cf(st2_out, g2r, be2r, a2bc, b2bc2)
            for lt in range(NTF):
                tmp = misc.tile([P, D], f32, tag="o1")
                nc.vector.tensor_mul(tmp[:], x2res[:, lt, :], a2bc[:])
                ot = misc.tile([P, D], f32, tag="ot")
                if lt % 2 == 0:
                    nc.vector.tensor_add(ot[:], tmp[:], b2bc2[:])
                else:
                    nc.gpsimd.tensor_tensor(
                        out=ot[:], in0=tmp[:], in1=b2bc2[:], op=AluOp.add
                    )
                nc.sync.dma_start(
                    out=out_d[lt * P : (lt + 1) * P, :], in_=ot[:]
                )

    nc.compile()
    return nc


# ----------------------------------------------------------------- entry


def _run(inputs, trace=False, tmpdir=None):
    from concourse.bass_utils import run_bass_kernel_spmd

    H_src = np.asarray(inputs["H_src"], dtype=np.float32)
    target_emb = np.asarray(inputs["target_emb"], dtype=np.float32)
    W_fwd = np.asarray(inputs["W_fwd"], dtype=np.float32)
    b_fwd = np.asarray(inputs["b_fwd"], dtype=np.float32)
    W_bwd = np.asarray(inputs["W_bwd"], dtype=np.float32)
    b_bwd = np.asarray(inputs["b_bwd"], dtype=np.float32)
    gamma = np.asarray(inputs["gamma"], dtype=np.float32)
    beta = np.asarray(inputs["beta"], dtype=np.float32)
    vals = np.asarray(inputs["vals"], dtype=np.float32)
    rows = np.asarray(inputs["rows"]).astype(np.int64)
    cols = np.asarray(inputs["cols"]).astype(np.int64)

    n_src, D = H_src.shape
    n_tgt = target_emb.shape[0]
    assert D == D_FIXED and n_tgt == N_TGT and n_src == N_SRC

    fwd, bwd, idxB, valB, idxE, valE, idxG, valG = _make_plans(rows, cols, vals)

    ncolsF_core = fwd.ncols
    ncolsB_core = bwd.ncols
    nc = _build_program(list(ncolsF_core), list(ncolsB_core))

    # per-core permuted emb rows / output slots
    part = fwd.part  # [256, 128] target ids (post-swap)
    in_maps = []
    perms = []
    for c in range(NCORES):
        tiles = part[c::NCORES]              # [NTF, 128] lt-major
        perm = tiles.reshape(-1)
        perms.append(perm)
        in_maps.append(
            {
                "Hsrc": H_src,
                "emb": np.ascontiguousarray(target_emb[perm]),
                "W0": W_fwd[0],
                "Wb": W_bwd[0],
                "W1": W_fwd[1],
                "b0": b_fwd[0].reshape(1, D),
                "bb": b_bwd[0].reshape(1, D),
                "b1": b_fwd[1].reshape(1, D),
                "g1": gamma[0].reshape(1, D),
                "be1": beta[0].reshape(1, D),
                "g2": gamma[1].reshape(1, D),
                "be2": beta[1].reshape(1, D),
                "fe_i16": idxB[c],
                "fe_val": valB[c],
                "be_i16": idxE[c],
                "be_val": valE[c],
                "ge_i16": idxG[c],
                "ge_val": valG[c],
            }
        )

    res = run_bass_kernel_spmd(
        nc, in_maps, list(range(NCORES)), trace=trace, tmpdir=tmpdir
    )
    out = np.empty((N_TGT, D), np.float32)
    for c in range(NCORES):
        out[perms[c]] = np.asarray(res.results[c]["out"]).astype(np.float32)
    return out, res


def kernel(**inputs) -> np.ndarray:
    out, _ = _run(inputs)
    return out
